# revision 1
# baseline (speedup 1.0000x reference)
"""Trainium2 Bass kernel for nn_GATv2GCN22 (4-relation GATv2 x2 + GraphConv x2).

Sharding: 8 cores; core c handles relation c//2, destination-node half c%2.
Within a relation pair, halves exchange node features between layers via
pair AllGather collectives.

Per GAT layer on each core:
  dense:  xl = h @ Wl, xr = h @ Wr over the full padded node table (PE, f32r)
  edge :  per 128-dst-node window (uniform T chunks of 128 sorted-by-dst
          edges): dma_gather xl[src] and xr[dst]; z = leaky(G + XR);
          e = reduce(z * att); p = exp(e); one-hot Se from dst-locals;
          s = sum_e p (PE); Gw = G * p (bf16); agg^T += Gw^T_chunk Se_chunk
          (PE, feature-major); h = relu(agg / s + b), PE-transposed to
          node-major, written to the half table.
GraphConv layers reuse the same windows with self-loops masked out of the
one-hot, plus a fused dense epilogue per window.
"""
import numpy as np
import concourse.bacc as bacc
import concourse.tile as tile
import concourse.mybir as mybir
import concourse.bass as bass
from concourse.bass import ds
from concourse.bass_utils import run_bass_kernel_spmd

F32 = mybir.dt.float32
F32R = mybir.dt.float32r
BF16 = mybir.dt.bfloat16
I16 = mybir.dt.int16
AF = mybir.ActivationFunctionType
OP = mybir.AluOpType
AX = mybir.AxisListType

N = 20000
E = 320000
R = 4
H = 4
HID = 64
DIN = 256
OUT = 64
NEG = 0.2
NCORES = 8
P = 128

_CACHE = {}


def _cfg(pair_split):
    nh = N // 2 if pair_split else N          # real nodes handled per core
    nw = -(-((nh + P - 1) // P) // 4) * 4     # 128-node windows, 4-aligned
    th = nw * P                               # padded half-table height
    tt = 2 * th if pair_split else th         # full gather-table height
    return nh, nw, th, tt


def _build_nc(T, pair_split):
    nh, NW, TH, TT = _cfg(pair_split)
    nc = bacc.Bacc("TRN2", target_bir_lowering=False, debug=False,
                   num_devices=NCORES)

    def inp(name, shape, dt=F32):
        return nc.dram_tensor(name, shape, dt, kind="ExternalInput").ap()

    # graph structure (shared by all 4 layers; rows stride P per window)
    src_rows = inp("src_rows", [NW * P, 8 * T], I16)
    dst_rows = inp("dst_rows", [NW * P, 8 * T], I16)
    dstc12 = inp("dstc12", [NW * P, T])
    dstc34 = inp("dstc34", [NW * P, T])
    xT = inp("xT", [2, P, TT], BF16)
    Wl1 = inp("Wl1", [2, P, DIN], BF16)
    Wr1 = inp("Wr1", [2, P, DIN], BF16)
    Wl2 = inp("Wl2", [2, P, DIN], BF16)
    Wr2 = inp("Wr2", [2, P, DIN], BF16)
    att1 = inp("att1", [P, 1, H, HID])
    att2 = inp("att2", [P, 1, H, HID])
    b1c = inp("b1c", [P, 2, 1])
    b2c = inp("b2c", [P, 2, 1])
    Wrel3 = inp("Wrel3", [2, P, HID], BF16)
    Wroot3 = inp("Wroot3", [2, P, HID], BF16)
    Wrel4 = inp("Wrel4", [HID, OUT], BF16)
    Wroot4 = inp("Wroot4", [HID, OUT], BF16)
    b3r = inp("b3r", [P, HID])
    b4r = inp("b4r", [P, OUT])
    iota = inp("iota", [P, 1, P])
    ident = inp("ident", [P, P])
    identb = inp("identb", [P, P], BF16)
    rsel = inp("rsel", [H, 2, P])
    out = nc.dram_tensor("out", [TH, OUT], F32, kind="ExternalOutput").ap()

    groups = [[0, 1], [2, 3], [4, 5], [6, 7]]

    with tile.TileContext(nc) as tc:
        with tc.tile_pool(name="dram", bufs=1, space="DRAM") as dram:
            t_xl = dram.tile([TT, DIN], BF16, name="t_xl")
            t_xr = dram.tile([TT, DIN], BF16, name="t_xr")
            t_b1 = dram.tile([TH, DIN], BF16, name="t_b1")
            t_h1 = (dram.tile([TT, DIN], BF16, name="t_h1")
                    if pair_split else t_b1)
            t_b2 = dram.tile([TH, DIN], BF16, name="t_b2")
            t_h2 = (dram.tile([TT, DIN], BF16, name="t_h2")
                    if pair_split else t_b2)
            t_b3 = dram.tile([TH, HID], F32, name="t_b3")
            t_h3 = (dram.tile([TT, HID], F32, name="t_h3")
                    if pair_split else t_b3)

            # ---------- constants resident in SBUF ----------
            with tc.tile_pool(name="const", bufs=1) as cpool:
                def const2(name, src, shape, dt=F32):
                    # src [2, P, X] -> tile [P, 2, X]
                    t = cpool.tile(shape, dt, tag=name)
                    for k in range(2):
                        nc.sync.dma_start(t[:, k], src[k])
                    return t

                def const1(name, src, shape, dt=F32):
                    t = cpool.tile(shape, dt, tag=name)
                    nc.sync.dma_start(t[:], src)
                    return t

                iota_t = const1("iota", iota[:], [P, 1, P])
                id_t = const1("ident", ident[:], [P, P])
                idb_t = const1("identb", identb[:], [P, P], BF16)
                rsel_t = const1("rsel", rsel[:], [H, 2, P])
                att1_t = const1("att1", att1[:], [P, 1, H, HID])
                att2_t = const1("att2", att2[:], [P, 1, H, HID])
                b1_t = const1("b1", b1c[:], [P, 2, 1])
                b2_t = const1("b2", b2c[:], [P, 2, 1])
                b3_t = const1("b3", b3r[:], [P, HID])
                b4_t = const1("b4", b4r[:], [P, OUT])
                w3l_t = const2("w3l", Wrel3, [P, 2, HID], BF16)
                w3r_t = const2("w3r", Wroot3, [P, 2, HID], BF16)
                w4l_t = const1("w4l", Wrel4[:], [HID, OUT], BF16)
                w4r_t = const1("w4r", Wroot4[:], [HID, OUT], BF16)

                # ================= phases =================

                def dense(src_h, Wl_ap, Wr_ap):
                    """xl/xr tables for all TT rows; 512-node blocks."""
                    with (
                        tc.tile_pool(name="dsb", bufs=3) as sb,
                        tc.tile_pool(name="dps", bufs=2, space="PSUM") as ps,
                        tc.tile_pool(name="dwp", bufs=1) as wp,
                    ):
                        wl_t = wp.tile([P, 2, DIN], BF16, tag="wl")
                        wr_t = wp.tile([P, 2, DIN], BF16, tag="wr")
                        for k in range(2):
                            nc.sync.dma_start(wl_t[:, k], Wl_ap[k])
                            nc.sync.dma_start(wr_t[:, k], Wr_ap[k])

                        def body(iv):
                            for s in range(4):
                                off = ds(iv + s * P, P)
                                lh = sb.tile([P, 2, P], BF16, tag="lh")
                                if src_h is None:
                                    for k in range(2):
                                        nc.sync.dma_start(lh[:, k],
                                                          xT[k, :, off])
                                else:
                                    hn = sb.tile([P, DIN], BF16, tag="hn")
                                    nc.sync.dma_start(hn[:], src_h[off, :])
                                    lhp = ps.tile([P, 2, P], BF16, tag="lhp")
                                    for k in range(2):
                                        nc.tensor.transpose(
                                            lhp[:, k], hn[:, ds(k * P, P)],
                                            idb_t[:])
                                    for k in range(2):
                                        nc.vector.tensor_copy(lh[:, k],
                                                              lhp[:, k])
                                xlp = ps.tile([P, DIN], F32, tag="xlp")
                                xrp = ps.tile([P, DIN], F32, tag="xrp")
                                for k in range(2):
                                    nc.tensor.matmul(
                                        xlp[:], lh[:, k], wl_t[:, k],
                                        start=(k == 0), stop=(k == 1))
                                for k in range(2):
                                    nc.tensor.matmul(
                                        xrp[:], lh[:, k], wr_t[:, k],
                                        start=(k == 0), stop=(k == 1))
                                xls = sb.tile([P, DIN], BF16, tag="xls")
                                nc.vector.tensor_copy(xls[:], xlp[:])
                                xrs = sb.tile([P, DIN], BF16, tag="xrs")
                                nc.scalar.copy(xrs[:], xrp[:])
                                nc.sync.dma_start(t_xl[off, :], xls[:])
                                nc.sync.dma_start(t_xr[off, :], xrs[:])

                        tc.For_i_unrolled(0, TT, 4 * P, body, max_unroll=2)

                def gat_edge(att_t, b_t, t_dst, dst_dt):
                    import os
                    kedge = int(os.environ.get("KEDGE", "10"))
                    with (
                        tc.tile_pool(name="esb", bufs=2) as sb,
                        tc.tile_pool(name="eps", bufs=2, space="PSUM") as ps,
                        tc.tile_pool(name="eps1", bufs=1, space="PSUM") as ps1,
                    ):
                        def body(iv):
                            rows = ds(iv, P)
                            isx = sb.tile([P, 8 * T], I16, tag="isx")
                            nc.sync.dma_start(isx[:], src_rows[rows, :])
                            idx = sb.tile([P, 8 * T], I16, tag="idx")
                            nc.sync.dma_start(idx[:], dst_rows[rows, :])
                            dstc = sb.tile([P, T, 1], F32, tag="dstc")
                            nc.sync.dma_start(dstc[:, :, 0], dstc12[rows, :])

                            if kedge < 1:
                                tmp0 = sb.tile([P, T], F32, tag="tmp0")
                                nc.vector.tensor_copy(tmp0[:], dstc[:, :, 0])
                                nc.sync.dma_start(t_dst[rows, 0:T], tmp0[:])
                                return
                            G = sb.tile([P, T, DIN], BF16, tag="G")
                            nc.gpsimd.dma_gather(
                                out_ap=G[:], in_ap=t_xl[:, :], idxs_ap=isx[:],
                                num_idxs=T * P, num_idxs_reg=T * P,
                                elem_size=DIN, single_packet=False)
                            if kedge < 2:
                                nc.sync.dma_start(t_dst[rows, :], G[:, 0, :])
                                return
                            XR = sb.tile([P, T, DIN], BF16, tag="XR")
                            nc.gpsimd.dma_gather(
                                out_ap=XR[:], in_ap=t_xr[:, :], idxs_ap=idx[:],
                                num_idxs=T * P, num_idxs_reg=T * P,
                                elem_size=DIN, single_packet=False)
                            if kedge < 3:
                                nc.sync.dma_start(t_dst[rows, :], XR[:, 0, :])
                                return
                            # z = leaky(G + XR), in place in XR
                            nc.vector.tensor_add(XR[:], G[:], XR[:])
                            nc.vector.scalar_tensor_tensor(
                                out=XR[:], in0=XR[:], scalar=NEG, in1=XR[:],
                                op0=OP.mult, op1=OP.max)
                            if kedge < 4:
                                nc.sync.dma_start(t_dst[rows, :], XR[:, 0, :])
                                return
                            # e = reduce(z * att); p = exp(e)
                            z4 = XR[:].rearrange("p t (h c) -> p t h c", h=H)
                            nc.vector.tensor_tensor(
                                out=z4, in0=z4,
                                in1=att_t[:].broadcast_to([P, T, H, HID]),
                                op=OP.mult)
                            pf = sb.tile([P, T, H, 1], F32, tag="pf")
                            nc.vector.tensor_reduce(
                                out=pf[:, :, :, 0], in_=z4, axis=AX.X,
                                op=OP.add)
                            nc.scalar.activation(pf[:], pf[:], AF.Exp)
                            pb = sb.tile([P, T, H], BF16, tag="pb")
                            nc.vector.tensor_copy(pb[:], pf[:, :, :, 0])
                            if kedge < 4:
                                nc.sync.dma_start(t_dst[rows, 0:T],
                                                  pf[:, :, 0, 0])
                                return
                            # one-hot Se[e, n] = (dstc[e] == n)
                            se = sb.tile([P, T, P], BF16, tag="se")
                            nc.vector.tensor_tensor(
                                out=se[:],
                                in0=dstc[:].broadcast_to([P, T, P]),
                                in1=iota_t[:].broadcast_to([P, T, P]),
                                op=OP.is_equal)
                            if kedge < 5:
                                tmp4 = sb.tile([P, P], F32, tag="tmp4")
                                nc.vector.tensor_copy(tmp4[:], se[:, 0, :])
                                nc.sync.dma_start(t_dst[rows, 0:P], tmp4[:])
                                return
                            # s[h, n] = sum_e p
                            sp = ps1.tile([H, P], F32, tag="sp")
                            for j in range(T):
                                nc.tensor.matmul(
                                    sp[:], pb[:, j], se[:, j],
                                    start=(j == 0), stop=(j == T - 1))
                            srec = sb.tile([H, P], F32, tag="srec")
                            nc.vector.tensor_scalar(
                                out=srec[:], in0=sp[:], scalar1=1e-30,
                                scalar2=None, op0=OP.add)
                            nc.vector.reciprocal(srec[:], srec[:])
                            if kedge < 6:
                                nc.sync.dma_start(t_dst[ds(iv, H), 0:P],
                                                  srec[:])
                                return
                            # Gw = G * p (bf16)
                            gw = sb.tile([P, T, H, HID], BF16, tag="gw")
                            nc.vector.tensor_tensor(
                                out=gw[:],
                                in0=G[:].rearrange("p t (h c) -> p t h c",
                                                   h=H),
                                in1=pf[:].broadcast_to([P, T, H, HID]),
                                op=OP.mult)
                            gw2 = gw[:].rearrange("p t h c -> p t (h c)")
                            if kedge < 7:
                                tmp6 = sb.tile([P, HID], F32, tag="tmp6")
                                nc.vector.tensor_copy(tmp6[:], gw[:, 0, 0, :])
                                nc.sync.dma_start(t_dst[rows, 0:HID], tmp6[:])
                                return
                            # agg^T[f, n] += Gw_chunk^T @ Se_chunk
                            agg = ps.tile([P, 2, P], F32, tag="agg")
                            for k in range(2):
                                for j in range(T):
                                    nc.tensor.matmul(
                                        agg[:, k], gw2[:, j, ds(k * P, P)],
                                        se[:, j], start=(j == 0),
                                        stop=(j == T - 1))
                            if kedge < 8:
                                tmp7 = sb.tile([P, P], F32, tag="tmp7")
                                nc.vector.tensor_copy(tmp7[:], agg[:, 0])
                                nc.sync.dma_start(t_dst[rows, 0:P], tmp7[:])
                                return
                            # rb[f, n] = 1/s(head(f), n)
                            rb = ps1.tile([P, 2, P], F32, tag="rb")
                            for k in range(2):
                                nc.tensor.matmul(rb[:, k], rsel_t[:, k],
                                                 srec[:], start=True,
                                                 stop=True)
                            rbs = sb.tile([P, 2, P], F32, tag="rbs")
                            nc.scalar.copy(rbs[:], rb[:])
                            if kedge < 9:
                                nc.sync.dma_start(t_dst[rows, 0:P],
                                                  rbs[:, 0])
                                return
                            hT = sb.tile([P, 2, P], F32, tag="hT")
                            nc.vector.tensor_mul(hT[:], agg[:], rbs[:])
                            for k in range(2):
                                nc.scalar.activation(hT[:, k], hT[:, k],
                                                     AF.Relu, bias=b_t[:, k])
                            if kedge < 10:
                                nc.sync.dma_start(t_dst[rows, 0:P],
                                                  hT[:, 0])
                                return
                            hp = ps.tile([P, 2, P], F32, tag="hp")
                            for k in range(2):
                                nc.tensor.transpose(hp[:, k], hT[:, k],
                                                    id_t[:])
                            hn = sb.tile([P, DIN], dst_dt, tag="hn")
                            for k in range(2):
                                nc.vector.tensor_copy(hn[:, ds(k * P, P)],
                                                      hp[:, k])
                            nc.sync.dma_start(t_dst[rows, :], hn[:])

                        tc.For_i_unrolled(0, NW * P, P, body, max_unroll=4)

                def gconv(t_gsrc, t_hown, wl_sl, wr_sl, b_t, t_dst, hid_out,
                          src_din, last):
                    """agg = sum h[src]; out = relu?(agg@Wl + h@Wr + b)."""
                    gdt = BF16 if src_din == DIN else F32
                    kch = max(src_din // P, 1)
                    mpart = P if kch > 1 else src_din
                    idt = idb_t if gdt == BF16 else id_t
                    with (
                        tc.tile_pool(name="gsb", bufs=2) as sb,
                        tc.tile_pool(name="gps", bufs=2, space="PSUM") as ps,
                        tc.tile_pool(name="gps1", bufs=1, space="PSUM") as ps1,
                    ):
                        def body(iv):
                            rows = ds(iv, P)
                            isx = sb.tile([P, 8 * T], I16, tag="isx")
                            nc.sync.dma_start(isx[:], src_rows[rows, :])
                            dstc = sb.tile([P, T, 1], F32, tag="dstc")
                            nc.sync.dma_start(dstc[:, :, 0], dstc34[rows, :])
                            G = sb.tile([P, T, src_din], gdt, tag="G")
                            nc.gpsimd.dma_gather(
                                out_ap=G[:], in_ap=t_gsrc[:, :],
                                idxs_ap=isx[:], num_idxs=T * P,
                                num_idxs_reg=T * P, elem_size=src_din,
                                single_packet=False)
                            se = sb.tile([P, T, P], BF16, tag="se")
                            nc.vector.tensor_tensor(
                                out=se[:],
                                in0=dstc[:].broadcast_to([P, T, P]),
                                in1=iota_t[:].broadcast_to([P, T, P]),
                                op=OP.is_equal)
                            if gdt == BF16:
                                gb = G
                            else:
                                gb = sb.tile([P, T, src_din], BF16, tag="gb")
                                nc.scalar.copy(gb[:], G[:])
                            agg = ps.tile([mpart, kch, P], F32, tag="agg")
                            for k in range(kch):
                                for j in range(T):
                                    nc.tensor.matmul(
                                        agg[:, k],
                                        gb[:, j, ds(k * P, P)] if kch > 1
                                        else gb[:, j],
                                        se[:, j], start=(j == 0),
                                        stop=(j == T - 1))
                            # fused dense epilogue
                            hw = sb.tile([P, src_din], gdt, tag="hw")
                            nc.sync.dma_start(hw[:], t_hown[rows, :])
                            hTp = ps1.tile([mpart, kch, P], gdt, tag="hTp")
                            for k in range(kch):
                                nc.tensor.transpose(
                                    hTp[:, k],
                                    hw[:, ds(k * P, P)] if kch > 1 else hw[:],
                                    idt[:])
                            aT = sb.tile([mpart, kch, P], BF16, tag="aT")
                            nc.vector.tensor_copy(aT[:], agg[:])
                            hT = sb.tile([mpart, kch, P], BF16, tag="hTt")
                            nc.vector.tensor_copy(hT[:], hTp[:])
                            op_ = ps.tile([P, hid_out], F32, tag="op")
                            for k in range(kch):
                                nc.tensor.matmul(op_[:], aT[:, k], wl_sl[k],
                                                 start=(k == 0), stop=False)
                            for k in range(kch):
                                nc.tensor.matmul(op_[:], hT[:, k], wr_sl[k],
                                                 start=False,
                                                 stop=(k == kch - 1))
                            os_ = sb.tile([P, hid_out], F32, tag="os")
                            nc.vector.tensor_add(os_[:], op_[:], b_t[:])
                            if not last:
                                nc.vector.tensor_scalar_max(os_[:], os_[:],
                                                            0.0)
                            nc.sync.dma_start(t_dst[rows, :], os_[:])

                        tc.For_i_unrolled(0, NW * P, P, body, max_unroll=4)

                def exchange(src_t, dst_t):
                    import os
                    if not pair_split or os.environ.get("KNOCC") == "1":
                        return
                    nc.gpsimd.collective_compute(
                        "AllGather", OP.bypass, replica_groups=groups,
                        ins=[src_t.opt()], outs=[dst_t.opt()])

                # ================= the network =================
                import os
                phases = int(os.environ.get("KPHASES", "7"))

                def network():
                    if phases >= 1:
                        dense(None, Wl1, Wr1)
                    if phases >= 2:
                        gat_edge(att1_t, b1_t, t_b1, BF16)
                    if phases >= 3:
                        exchange(t_b1, t_h1)
                    if phases >= 4:
                        dense(t_h1, Wl2, Wr2)
                    if phases >= 5:
                        gat_edge(att2_t, b2_t, t_b2, BF16)
                        exchange(t_b2, t_h2)
                    if phases >= 6:
                        gconv(t_h2, t_b2, [w3l_t[:, 0], w3l_t[:, 1]],
                              [w3r_t[:, 0], w3r_t[:, 1]], b3_t, t_b3, HID,
                              DIN, False)
                    if phases >= 7:
                        exchange(t_b3, t_h3)
                        gconv(t_h3, t_b3, [w4l_t[:]], [w4r_t[:]], b4_t, out,
                              OUT, HID, True)

                krep = int(os.environ.get("KREP", "1"))
                if krep > 1:
                    with tc.For_i(0, krep, 1):
                        network()
                else:
                    network()

    nc.compile()
    return nc


def _host_prep(x, edge_indices, Wl1, Wr1, att1, b1, Wl2, Wr2, att2, b2,
               Wrel3, Wroot3, b3, Wrel4, Wroot4, b4, pair_split=True):
    import ml_dtypes
    nh, NW, TH, TT = _cfg(pair_split)
    x = np.asarray(x, np.float32)
    ei = np.asarray(edge_indices)

    def glob_row(n):
        if pair_split:
            return np.where(n < nh, n, TH + (n - nh))
        return n

    structs = []
    chunk_counts = []
    for c in range(NCORES):
        r, half = c // 2, c % 2
        src, dst = ei[r, 0].astype(np.int64), ei[r, 1].astype(np.int64)
        if pair_split:
            m = (dst >= half * nh) & (dst < (half + 1) * nh)
            src, dst = src[m], dst[m] - half * nh
        loops_src = np.arange(nh) + (half * nh if pair_split else 0)
        s_all = np.concatenate([src, loops_src])
        d_all = np.concatenate([dst, np.arange(nh)])
        isloop = np.zeros(len(s_all), bool)
        isloop[len(src):] = True
        order = np.argsort(d_all, kind="stable")
        s_all, d_all, isloop = s_all[order], d_all[order], isloop[order]
        counts = np.bincount(d_all // P, minlength=NW)
        chunk_counts.append(np.ceil(counts / P).astype(int))
        structs.append((s_all, d_all, isloop, counts))

    T = int(max(cc.max() for cc in chunk_counts))

    # shared constants
    xpad = np.zeros((TT, DIN), np.float32)
    xpad[:nh] = x[:nh]
    if pair_split:
        xpad[TH:TH + nh] = x[nh:]
    xT = np.ascontiguousarray(xpad.T).reshape(2, P, TT)\
        .astype(ml_dtypes.bfloat16)
    iota_c = np.broadcast_to(np.arange(P, dtype=np.float32)[None, None],
                             (P, 1, P)).copy()
    ident_c = np.eye(P, dtype=np.float32)
    identb_c = np.eye(P, dtype=ml_dtypes.bfloat16)
    rsel_c = np.zeros((H, 2, P), np.float32)
    for k in range(2):
        for f in range(P):
            rsel_c[(k * P + f) // HID, k, f] = 1.0

    in_maps = []
    for c in range(NCORES):
        r, half = c // 2, c % 2
        s_all, d_all, isloop, counts = structs[c]
        srcr = np.zeros((NW, T * P), np.int64)
        dstr = np.zeros((NW, T * P), np.int64)
        dc12 = np.full((NW, T * P), -1.0, np.float32)
        dc34 = np.full((NW, T * P), -1.0, np.float32)
        start = 0
        off = half * TH if pair_split else 0
        for w in range(NW):
            cnt = counts[w]
            sl = slice(start, start + cnt)
            start += cnt
            srcr[w, :cnt] = glob_row(s_all[sl])
            dstr[w, :cnt] = off + d_all[sl]
            dl = (d_all[sl] - w * P).astype(np.float32)
            dc12[w, :cnt] = dl
            dc34[w, :cnt] = np.where(isloop[sl], -1.0, dl)

        def wrap(a):  # [NW, T*P] -> [NW*P, 8T] int16, replicated per Q7 core
            b = a.astype(np.int16).reshape(NW, 8 * T, 16)
            b = np.transpose(b, (0, 2, 1))
            return np.tile(b, (1, 8, 1)).reshape(NW * P, 8 * T).copy()

        def colmajor(a):  # [NW, T*P] -> [NW*P, T]
            return np.ascontiguousarray(
                np.transpose(a.reshape(NW, T, P), (0, 2, 1))).reshape(
                    NW * P, T)

        kchunk = lambda w: np.ascontiguousarray(
            np.asarray(w[r], np.float32)).reshape(2, P, -1)
        att_rep = lambda a: np.broadcast_to(
            np.asarray(a[r], np.float32)[None, None], (P, 1, H, HID)).copy()
        bcol = lambda b: np.asarray(b[r], np.float32).reshape(2, P, 1)\
            .transpose(1, 0, 2).copy()
        brep = lambda b, n: np.broadcast_to(
            np.asarray(b[r], np.float32)[None], (P, n)).copy()
        bf = lambda a: np.ascontiguousarray(a).astype(ml_dtypes.bfloat16)

        in_maps.append(dict(
            src_rows=wrap(srcr), dst_rows=wrap(dstr),
            dstc12=colmajor(dc12), dstc34=colmajor(dc34),
            xT=xT,
            Wl1=bf(kchunk(Wl1)), Wr1=bf(kchunk(Wr1)),
            Wl2=bf(kchunk(Wl2)), Wr2=bf(kchunk(Wr2)),
            att1=att_rep(att1), att2=att_rep(att2),
            b1c=bcol(b1), b2c=bcol(b2),
            Wrel3=bf(kchunk(Wrel3)), Wroot3=bf(kchunk(Wroot3)),
            Wrel4=bf(np.asarray(Wrel4[r], np.float32)),
            Wroot4=bf(np.asarray(Wroot4[r], np.float32)),
            b3r=brep(b3, HID), b4r=brep(b4, OUT),
            iota=iota_c, ident=ident_c, identb=identb_c, rsel=rsel_c,
        ))
    return in_maps, T


def kernel(x, edge_indices, Wl1, Wr1, att1, b1, Wl2, Wr2, att2, b2,
           Wrel3, Wroot3, b3, Wrel4, Wroot4, b4, pair_split=True):
    in_maps, T = _host_prep(x, edge_indices, Wl1, Wr1, att1, b1, Wl2, Wr2,
                            att2, b2, Wrel3, Wroot3, b3, Wrel4, Wroot4, b4,
                            pair_split)
    import os
    key = (T, pair_split, os.environ.get("KPHASES"), os.environ.get("KEDGE"),
           os.environ.get("KNOCC"), os.environ.get("KREP"))
    if key not in _CACHE:
        _CACHE[key] = _build_nc(T, pair_split)
    nc = _CACHE[key]

    res = run_bass_kernel_spmd(nc, in_maps, core_ids=list(range(NCORES)))

    nh, NW, TH, TT = _cfg(pair_split)
    outp = np.zeros((N, R, OUT), np.float32)
    for c in range(NCORES):
        r, half = c // 2, c % 2
        o = res.results[c]["out"]
        if pair_split:
            outp[half * nh:(half + 1) * nh, r] = o[:nh]
        elif half == 0:
            outp[:, r] = o[:N]
    return outp



# revision 13
# speedup vs baseline: 1.3813x; 1.3813x over previous
"""Trainium2 Bass kernel for nn_GATv2GCN22 (4-relation GATv2 x2 + GraphConv x2).

Sharding: 8 cores; core c handles relation c//2, destination-node half c%2.
Within a relation pair, halves exchange node features between layers via
pair AllGather collectives.

Per GAT layer on each core:
  dense:  xl = h @ Wl, xr = h @ Wr over the full padded node table (PE)
  edge :  per 128-dst-node window (uniform T chunks of 128 sorted-by-dst
          edges): dma_gather xl[src] and xr[dst]; z = leaky(G + XR);
          e = reduce(z * att); p = exp(e); one-hot Se from dst-locals
          (Pool engine); rhs = [G * p | p] (264 cols);
          agg[n, 0:256|256:260] += Se_chunk^T @ rhs_chunk (PE, node-major,
          numerator and softmax denominator in one accumulator);
          h = relu(agg[:, 0:256] / agg[:, 256:260] + b) written node-major.
GraphConv layers reuse the same windows with self-loops masked out of the
one-hot, plus a fused dense epilogue per window.

Timing support: _build_nc(dyn_rep=True) wraps the network in a For_i whose
trip count is read at runtime from the `krep` input tensor, so one NEFF
serves every repeat count (collectives are skipped in that build).
"""
import numpy as np
import concourse.bacc as bacc
import concourse.tile as tile
import concourse.mybir as mybir
import concourse.bass as bass
from concourse.bass import ds, RegisterHandles, make_scalar_value
from concourse.bass_utils import run_bass_kernel_spmd

F32 = mybir.dt.float32
BF16 = mybir.dt.bfloat16
I16 = mybir.dt.int16
I32 = mybir.dt.int32
AF = mybir.ActivationFunctionType
OP = mybir.AluOpType
AX = mybir.AxisListType

N = 20000
E = 320000
R = 4
H = 4
HID = 64
DIN = 256
OUT = 64
NEG = 0.2
NCORES = 8
P = 128

_CACHE = {}


def _cfg(pair_split):
    nh = N // 2 if pair_split else N          # real nodes handled per core
    nw = -(-((nh + P - 1) // P) // 4) * 4     # 128-node windows, 4-aligned
    th = nw * P                               # padded half-table height
    tt = 2 * th if pair_split else th         # full gather-table height
    return nh, nw, th, tt


def _build_nc(T, pair_split, dyn_rep=False):
    nh, NW, TH, TT = _cfg(pair_split)
    nc = bacc.Bacc("TRN2", target_bir_lowering=False, debug=False,
                   num_devices=NCORES)

    def inp(name, shape, dt=F32):
        return nc.dram_tensor(name, shape, dt, kind="ExternalInput").ap()

    # graph structure (shared by all 4 layers; rows stride P per window)
    src_rows = inp("src_rows", [NW * P, 8 * T], I16)
    dst_rows = inp("dst_rows", [NW * P, 8 * T], I16)
    dstc12 = inp("dstc12", [NW * P, T], BF16)
    dstc34 = inp("dstc34", [NW * P, T], BF16)
    xT = inp("xT", [2, P, TT], BF16)
    Wl1 = inp("Wl1", [2, P, DIN], BF16)
    Wr1 = inp("Wr1", [2, P, DIN], BF16)
    Wl2 = inp("Wl2", [2, P, DIN], BF16)
    Wr2 = inp("Wr2", [2, P, DIN], BF16)
    att1 = inp("att1", [P, 1, H, HID], BF16)
    att2 = inp("att2", [P, 1, H, HID], BF16)
    b1r = inp("b1r", [P, DIN])
    b2r = inp("b2r", [P, DIN])
    Wrel3 = inp("Wrel3", [2, P, HID], BF16)
    Wroot3 = inp("Wroot3", [2, P, HID], BF16)
    Wrel4 = inp("Wrel4", [HID, OUT], BF16)
    Wroot4 = inp("Wroot4", [HID, OUT], BF16)
    b3r = inp("b3r", [P, HID])
    b4r = inp("b4r", [P, OUT])
    iota = inp("iota", [P, 1, P], BF16)
    ident = inp("ident", [P, P])
    identb = inp("identb", [P, P], BF16)
    krep = inp("krep", [1, 1], I32) if dyn_rep else None
    out = nc.dram_tensor("out", [TH, OUT], F32, kind="ExternalOutput").ap()

    groups = [[0, 1], [2, 3], [4, 5], [6, 7]]

    with tile.TileContext(nc) as tc:
        with tc.tile_pool(name="dram", bufs=1, space="DRAM") as dram:
            t_xl = dram.tile([TT, DIN], BF16, name="t_xl")
            t_xr = dram.tile([TT, DIN], BF16, name="t_xr")
            t_b1 = dram.tile([TH, DIN], BF16, name="t_b1")
            t_h1 = (dram.tile([TT, DIN], BF16, name="t_h1")
                    if pair_split else t_b1)
            t_b2 = dram.tile([TH, DIN], BF16, name="t_b2")
            t_h2 = (dram.tile([TT, DIN], BF16, name="t_h2")
                    if pair_split else t_b2)
            t_b3 = dram.tile([TH, HID], F32, name="t_b3")
            t_h3 = (dram.tile([TT, HID], F32, name="t_h3")
                    if pair_split else t_b3)

            # ---------- constants resident in SBUF ----------
            with tc.tile_pool(name="const", bufs=1) as cpool:
                def const2(name, src, shape, dt=F32):
                    # src [2, P, X] -> tile [P, 2, X]
                    t = cpool.tile(shape, dt, tag=name)
                    for k in range(2):
                        nc.sync.dma_start(t[:, k], src[k])
                    return t

                def const1(name, src, shape, dt=F32):
                    t = cpool.tile(shape, dt, tag=name)
                    nc.sync.dma_start(t[:], src)
                    return t

                iota_t = const1("iota", iota[:], [P, 1, P], BF16)
                id_t = const1("ident", ident[:], [P, P])
                idb_t = const1("identb", identb[:], [P, P], BF16)
                att1_t = const1("att1", att1[:], [P, 1, H, HID], BF16)
                att2_t = const1("att2", att2[:], [P, 1, H, HID], BF16)
                b1_t = const1("b1", b1r[:], [P, DIN])
                b2_t = const1("b2", b2r[:], [P, DIN])
                b3_t = const1("b3", b3r[:], [P, HID])
                b4_t = const1("b4", b4r[:], [P, OUT])
                w3l_t = const2("w3l", Wrel3, [P, 2, HID], BF16)
                w3r_t = const2("w3r", Wroot3, [P, 2, HID], BF16)
                w4l_t = const1("w4l", Wrel4[:], [HID, OUT], BF16)
                w4r_t = const1("w4r", Wroot4[:], [HID, OUT], BF16)

                # ================= phases =================

                def dense(src_h, Wl_ap, Wr_ap):
                    """xl/xr tables for all TT rows; 512-node blocks."""
                    with (
                        tc.tile_pool(name="dsb", bufs=3) as sb,
                        tc.tile_pool(name="dps", bufs=2, space="PSUM") as ps,
                        tc.tile_pool(name="dwp", bufs=1) as wp,
                    ):
                        wl_t = wp.tile([P, 2, DIN], BF16, tag="wl")
                        wr_t = wp.tile([P, 2, DIN], BF16, tag="wr")
                        for k in range(2):
                            nc.sync.dma_start(wl_t[:, k], Wl_ap[k])
                            nc.sync.dma_start(wr_t[:, k], Wr_ap[k])

                        def body(iv):
                            for s in range(4):
                                off = ds(iv + s * P, P)
                                lh = sb.tile([P, 2, P], BF16, tag="lh")
                                if src_h is None:
                                    for k in range(2):
                                        nc.sync.dma_start(lh[:, k],
                                                          xT[k, :, off])
                                else:
                                    hn = sb.tile([P, DIN], BF16, tag="hn")
                                    nc.sync.dma_start(hn[:], src_h[off, :])
                                    lhp = ps.tile([P, 2, P], BF16, tag="lhp")
                                    for k in range(2):
                                        nc.tensor.transpose(
                                            lhp[:, k], hn[:, ds(k * P, P)],
                                            idb_t[:])
                                    for k in range(2):
                                        nc.vector.tensor_copy(lh[:, k],
                                                              lhp[:, k])
                                xlp = ps.tile([P, DIN], F32, tag="xlp")
                                xrp = ps.tile([P, DIN], F32, tag="xrp")
                                for k in range(2):
                                    nc.tensor.matmul(
                                        xlp[:], lh[:, k], wl_t[:, k],
                                        start=(k == 0), stop=(k == 1))
                                for k in range(2):
                                    nc.tensor.matmul(
                                        xrp[:], lh[:, k], wr_t[:, k],
                                        start=(k == 0), stop=(k == 1))
                                xls = sb.tile([P, DIN], BF16, tag="xls")
                                nc.vector.tensor_copy(xls[:], xlp[:])
                                xrs = sb.tile([P, DIN], BF16, tag="xrs")
                                nc.scalar.copy(xrs[:], xrp[:])
                                nc.sync.dma_start(t_xl[off, :], xls[:])
                                nc.sync.dma_start(t_xr[off, :], xrs[:])

                        tc.For_i_unrolled(0, TT, 4 * P, body, max_unroll=4)

                def gat_edge(att_t, b_t, t_dst):
                    import os
                    kedge = int(os.environ.get("KEDGE", "10"))
                    with (
                        tc.tile_pool(name="esb", bufs=3) as sb,
                        tc.tile_pool(name="esm", bufs=4) as sm,
                        tc.tile_pool(name="eps", bufs=3, space="PSUM") as ps,
                    ):
                        def body(iv):
                            rows = ds(iv, P)
                            isx = sm.tile([P, 8 * T], I16, tag="isx")
                            nc.sync.dma_start(isx[:], src_rows[rows, :])
                            idx = sm.tile([P, 8 * T], I16, tag="idx")
                            nc.sync.dma_start(idx[:], dst_rows[rows, :])
                            dstc = sm.tile([P, T, 1], BF16, tag="dstc")
                            nc.sync.dma_start(dstc[:, :, 0], dstc12[rows, :])

                            G = sb.tile([P, T, DIN], BF16, tag="G")
                            nc.gpsimd.dma_gather(
                                out_ap=G[:], in_ap=t_xl[:, :], idxs_ap=isx[:],
                                num_idxs=T * P, num_idxs_reg=T * P,
                                elem_size=DIN, single_packet=False)
                            if kedge < 2:
                                nc.sync.dma_start(t_dst[rows, :], G[:, 0, :])
                                return
                            XR = sb.tile([P, T, DIN], BF16, tag="XR")
                            nc.gpsimd.dma_gather(
                                out_ap=XR[:], in_ap=t_xr[:, :], idxs_ap=idx[:],
                                num_idxs=T * P, num_idxs_reg=T * P,
                                elem_size=DIN, single_packet=False)
                            # one-hot Se[e, n] = (dstc[e] == n)
                            se = sb.tile([P, T, P], BF16, tag="se")
                            nc.vector.tensor_tensor(
                                out=se[:],
                                in0=dstc[:].broadcast_to([P, T, P]),
                                in1=iota_t[:].broadcast_to([P, T, P]),
                                op=OP.is_equal)
                            if kedge < 3:
                                nc.sync.dma_start(t_dst[rows, :], XR[:, 0, :])
                                return
                            # z = leaky(G + XR), in place in XR
                            nc.vector.tensor_add(XR[:], G[:], XR[:])
                            nc.vector.scalar_tensor_tensor(
                                out=XR[:], in0=XR[:], scalar=NEG, in1=XR[:],
                                op0=OP.mult, op1=OP.max)
                            if kedge < 4:
                                nc.sync.dma_start(t_dst[rows, :], XR[:, 0, :])
                                return
                            # e = reduce(z * att); p = exp(e)
                            z4 = XR[:].rearrange("p t (h c) -> p t h c", h=H)
                            nc.vector.tensor_tensor(
                                out=z4, in0=z4,
                                in1=att_t[:].broadcast_to([P, T, H, HID]),
                                op=OP.mult)
                            pf = sm.tile([P, T, H, 1], F32, tag="pf")
                            nc.vector.tensor_reduce(
                                out=pf[:, :, :, 0], in_=z4, axis=AX.X,
                                op=OP.add)
                            nc.scalar.activation(pf[:], pf[:], AF.Exp)
                            if kedge < 5:
                                nc.sync.dma_start(t_dst[rows, 0:T],
                                                  pf[:, :, 0, 0])
                                return
                            # rhs = [G * p | p]  (264-wide, bf16)
                            gwp = sb.tile([P, T, 264], BF16, tag="gwp")
                            nc.vector.tensor_tensor(
                                out=gwp[:, :, 0:DIN].rearrange(
                                    "p t (h c) -> p t h c", h=H),
                                in0=G[:].rearrange("p t (h c) -> p t h c",
                                                   h=H),
                                in1=pf[:].broadcast_to([P, T, H, HID]),
                                op=OP.mult)
                            nc.vector.tensor_copy(gwp[:, :, DIN:DIN + H],
                                                  pf[:, :, :, 0])
                            if kedge < 6:
                                nc.sync.dma_start(t_dst[rows, :],
                                                  gwp[:, 0, 0:DIN])
                                return
                            # agg[n, 0:256] = sum_e p*G ; agg[n, 256:260] = s
                            agg = ps.tile([P, DIN + H], F32, tag="agg")
                            for j in range(T):
                                nc.tensor.matmul(
                                    agg[:], se[:, j], gwp[:, j, 0:DIN + H],
                                    start=(j == 0), stop=(j == T - 1))
                            if kedge < 7:
                                tmp7 = sm.tile([P, P], F32, tag="tmp7")
                                nc.vector.tensor_copy(tmp7[:], agg[:, 0:P])
                                nc.sync.dma_start(t_dst[rows, 0:P], tmp7[:])
                                return
                            # h = relu(agg / s + b), node-major
                            srec = sm.tile([P, H, 1], F32, tag="srec")
                            nc.vector.tensor_scalar(
                                out=srec[:, :, 0], in0=agg[:, DIN:DIN + H],
                                scalar1=1e-30, scalar2=None, op0=OP.add)
                            nc.vector.reciprocal(srec[:], srec[:])
                            hsc = sm.tile([P, H, HID], F32, tag="hsc")
                            nc.vector.tensor_tensor(
                                out=hsc[:],
                                in0=agg[:, 0:DIN].rearrange(
                                    "p (h c) -> p h c", h=H),
                                in1=srec[:].broadcast_to([P, H, HID]),
                                op=OP.mult)
                            nc.vector.tensor_add(
                                hsc[:].rearrange("p h c -> p (h c)"),
                                hsc[:].rearrange("p h c -> p (h c)"), b_t[:])
                            hb = sm.tile([P, DIN], BF16, tag="hb")
                            nc.scalar.activation(
                                hb[:], hsc[:].rearrange("p h c -> p (h c)"),
                                AF.Relu)
                            nc.sync.dma_start(t_dst[rows, :], hb[:])

                        tc.For_i_unrolled(0, NW * P, P, body, max_unroll=8)

                def gconv(t_gsrc, t_hown, wl_sl, wr_sl, b_t, t_dst, hid_out,
                          src_din, last):
                    """agg = sum h[src]; out = relu?(agg@Wl + h@Wr + b)."""
                    gdt = BF16 if src_din == DIN else F32
                    kch = max(src_din // P, 1)
                    mpart = P if kch > 1 else src_din
                    idt = idb_t if gdt == BF16 else id_t
                    with (
                        tc.tile_pool(name="gsb", bufs=3) as sb,
                        tc.tile_pool(name="gps", bufs=2, space="PSUM") as ps,
                        tc.tile_pool(name="gps1", bufs=2, space="PSUM") as ps1,
                    ):
                        def body(iv):
                            rows = ds(iv, P)
                            isx = sb.tile([P, 8 * T], I16, tag="isx")
                            nc.sync.dma_start(isx[:], src_rows[rows, :])
                            dstc = sb.tile([P, T, 1], BF16, tag="dstc")
                            nc.sync.dma_start(dstc[:, :, 0], dstc34[rows, :])
                            G = sb.tile([P, T, src_din], gdt, tag="G")
                            nc.gpsimd.dma_gather(
                                out_ap=G[:], in_ap=t_gsrc[:, :],
                                idxs_ap=isx[:], num_idxs=T * P,
                                num_idxs_reg=T * P, elem_size=src_din,
                                single_packet=False)
                            se = sb.tile([P, T, P], BF16, tag="se")
                            nc.vector.tensor_tensor(
                                out=se[:],
                                in0=dstc[:].broadcast_to([P, T, P]),
                                in1=iota_t[:].broadcast_to([P, T, P]),
                                op=OP.is_equal)
                            if gdt == BF16:
                                gb = G
                            else:
                                gb = sb.tile([P, T, src_din], BF16, tag="gb")
                                nc.scalar.copy(gb[:], G[:])
                            agg = ps.tile([mpart, kch, P], F32, tag="agg")
                            for k in range(kch):
                                for j in range(T):
                                    nc.tensor.matmul(
                                        agg[:, k],
                                        gb[:, j, ds(k * P, P)] if kch > 1
                                        else gb[:, j],
                                        se[:, j], start=(j == 0),
                                        stop=(j == T - 1))
                            # fused dense epilogue
                            hw = sb.tile([P, src_din], gdt, tag="hw")
                            nc.sync.dma_start(hw[:], t_hown[rows, :])
                            hTp = ps1.tile([mpart, kch, P], gdt, tag="hTp")
                            for k in range(kch):
                                nc.tensor.transpose(
                                    hTp[:, k],
                                    hw[:, ds(k * P, P)] if kch > 1 else hw[:],
                                    idt[:])
                            aT = sb.tile([mpart, kch, P], BF16, tag="aT")
                            nc.vector.tensor_copy(aT[:], agg[:])
                            hT = sb.tile([mpart, kch, P], BF16, tag="hTt")
                            nc.vector.tensor_copy(hT[:], hTp[:])
                            op_ = ps.tile([P, hid_out], F32, tag="op")
                            for k in range(kch):
                                nc.tensor.matmul(op_[:], aT[:, k], wl_sl[k],
                                                 start=(k == 0), stop=False)
                            for k in range(kch):
                                nc.tensor.matmul(op_[:], hT[:, k], wr_sl[k],
                                                 start=False,
                                                 stop=(k == kch - 1))
                            os_ = sb.tile([P, hid_out], F32, tag="os")
                            nc.vector.tensor_add(os_[:], op_[:], b_t[:])
                            if not last:
                                nc.vector.tensor_scalar_max(os_[:], os_[:],
                                                            0.0)
                            nc.sync.dma_start(t_dst[rows, :], os_[:])

                        tc.For_i_unrolled(0, NW * P, P, body, max_unroll=8)

                def exchange(src_t, dst_t):
                    import os
                    if (not pair_split or dyn_rep
                            or os.environ.get("KNOCC") == "1"):
                        return
                    nc.gpsimd.collective_compute(
                        "AllGather", OP.bypass, replica_groups=groups,
                        ins=[src_t.opt()], outs=[dst_t.opt()])

                # ================= the network =================
                import os
                phases = int(os.environ.get("KPHASES", "7"))

                def network():
                    if phases >= 1:
                        dense(None, Wl1, Wr1)
                    if phases >= 2:
                        gat_edge(att1_t, b1_t, t_b1)
                    if phases >= 3:
                        exchange(t_b1, t_h1)
                    if phases >= 4:
                        dense(t_h1, Wl2, Wr2)
                    if phases >= 5:
                        gat_edge(att2_t, b2_t, t_b2)
                        exchange(t_b2, t_h2)
                    if phases >= 6:
                        gconv(t_h2, t_b2, [w3l_t[:, 0], w3l_t[:, 1]],
                              [w3r_t[:, 0], w3r_t[:, 1]], b3_t, t_b3, HID,
                              DIN, False)
                    if phases >= 7:
                        exchange(t_b3, t_h3)
                        gconv(t_h3, t_b3, [w4l_t[:]], [w4r_t[:]], b4_t, out,
                              OUT, HID, True)

                if dyn_rep:
                    regs = []
                    for e in mybir.ALL_ENGINES:
                        eng = nc.engines[e]
                        r = eng.alloc_register(f"krep_{e.name}")
                        eng.reg_load(r, krep[0:1, 0:1])
                        regs.append(r)
                    kval = make_scalar_value(RegisterHandles(regs),
                                             min_val=0, max_val=1 << 20)
                    with tc.For_i(0, kval, 1):
                        network()
                else:
                    krep_env = int(os.environ.get("KREP", "1"))
                    if krep_env > 1:
                        with tc.For_i(0, krep_env, 1):
                            network()
                    else:
                        network()

    nc.compile()
    return nc


def _host_prep(x, edge_indices, Wl1, Wr1, att1, b1, Wl2, Wr2, att2, b2,
               Wrel3, Wroot3, b3, Wrel4, Wroot4, b4, pair_split=True):
    import ml_dtypes
    nh, NW, TH, TT = _cfg(pair_split)
    x = np.asarray(x, np.float32)
    ei = np.asarray(edge_indices)

    def glob_row(n):
        if pair_split:
            return np.where(n < nh, n, TH + (n - nh))
        return n

    structs = []
    chunk_counts = []
    for c in range(NCORES):
        r, half = c // 2, c % 2
        src, dst = ei[r, 0].astype(np.int64), ei[r, 1].astype(np.int64)
        if pair_split:
            m = (dst >= half * nh) & (dst < (half + 1) * nh)
            src, dst = src[m], dst[m] - half * nh
        loops_src = np.arange(nh) + (half * nh if pair_split else 0)
        s_all = np.concatenate([src, loops_src])
        d_all = np.concatenate([dst, np.arange(nh)])
        isloop = np.zeros(len(s_all), bool)
        isloop[len(src):] = True
        order = np.argsort(d_all, kind="stable")
        s_all, d_all, isloop = s_all[order], d_all[order], isloop[order]
        counts = np.bincount(d_all // P, minlength=NW)
        chunk_counts.append(np.ceil(counts / P).astype(int))
        structs.append((s_all, d_all, isloop, counts))

    T = int(max(cc.max() for cc in chunk_counts))

    # shared constants
    xpad = np.zeros((TT, DIN), np.float32)
    xpad[:nh] = x[:nh]
    if pair_split:
        xpad[TH:TH + nh] = x[nh:]
    xT = np.ascontiguousarray(xpad.T).reshape(2, P, TT)\
        .astype(ml_dtypes.bfloat16)
    iota_c = np.broadcast_to(
        np.arange(P, dtype=ml_dtypes.bfloat16)[None, None], (P, 1, P)).copy()
    ident_c = np.eye(P, dtype=np.float32)
    identb_c = np.eye(P, dtype=ml_dtypes.bfloat16)

    in_maps = []
    for c in range(NCORES):
        r, half = c // 2, c % 2
        s_all, d_all, isloop, counts = structs[c]
        srcr = np.zeros((NW, T * P), np.int64)
        dstr = np.zeros((NW, T * P), np.int64)
        dc12 = np.full((NW, T * P), -1.0, np.float32)
        dc34 = np.full((NW, T * P), -1.0, np.float32)
        start = 0
        off = half * TH if pair_split else 0
        for w in range(NW):
            cnt = counts[w]
            sl = slice(start, start + cnt)
            start += cnt
            srcr[w, :cnt] = glob_row(s_all[sl])
            dstr[w, :cnt] = off + d_all[sl]
            dl = (d_all[sl] - w * P).astype(np.float32)
            dc12[w, :cnt] = dl
            dc34[w, :cnt] = np.where(isloop[sl], -1.0, dl)

        def wrap(a):  # [NW, T*P] -> [NW*P, 8T] int16, replicated per Q7 core
            b = a.astype(np.int16).reshape(NW, 8 * T, 16)
            b = np.transpose(b, (0, 2, 1))
            return np.tile(b, (1, 8, 1)).reshape(NW * P, 8 * T).copy()

        def colmajor(a):  # [NW, T*P] -> [NW*P, T]
            return np.ascontiguousarray(
                np.transpose(a.reshape(NW, T, P), (0, 2, 1))).reshape(
                    NW * P, T)

        kchunk = lambda w: np.ascontiguousarray(
            np.asarray(w[r], np.float32)).reshape(2, P, -1)
        att_rep = lambda a: np.broadcast_to(
            np.asarray(a[r], np.float32).astype(ml_dtypes.bfloat16)
            [None, None], (P, 1, H, HID)).copy()
        brep = lambda b, n: np.broadcast_to(
            np.asarray(b[r], np.float32)[None], (P, n)).copy()
        bf = lambda a: np.ascontiguousarray(a).astype(ml_dtypes.bfloat16)

        in_maps.append(dict(
            src_rows=wrap(srcr), dst_rows=wrap(dstr),
            dstc12=colmajor(dc12).astype(ml_dtypes.bfloat16),
            dstc34=colmajor(dc34).astype(ml_dtypes.bfloat16),
            xT=xT,
            Wl1=bf(kchunk(Wl1)), Wr1=bf(kchunk(Wr1)),
            Wl2=bf(kchunk(Wl2)), Wr2=bf(kchunk(Wr2)),
            att1=att_rep(att1), att2=att_rep(att2),
            b1r=brep(b1, DIN), b2r=brep(b2, DIN),
            Wrel3=bf(kchunk(Wrel3)), Wroot3=bf(kchunk(Wroot3)),
            Wrel4=bf(np.asarray(Wrel4[r], np.float32)),
            Wroot4=bf(np.asarray(Wroot4[r], np.float32)),
            b3r=brep(b3, HID), b4r=brep(b4, OUT),
            iota=iota_c, ident=ident_c, identb=identb_c,
        ))
    return in_maps, T


def kernel(x, edge_indices, Wl1, Wr1, att1, b1, Wl2, Wr2, att2, b2,
           Wrel3, Wroot3, b3, Wrel4, Wroot4, b4, pair_split=True):
    in_maps, T = _host_prep(x, edge_indices, Wl1, Wr1, att1, b1, Wl2, Wr2,
                            att2, b2, Wrel3, Wroot3, b3, Wrel4, Wroot4, b4,
                            pair_split)
    import os
    key = (T, pair_split, os.environ.get("KPHASES"), os.environ.get("KEDGE"),
           os.environ.get("KNOCC"), os.environ.get("KREP"))
    if key not in _CACHE:
        _CACHE[key] = _build_nc(T, pair_split)
    nc = _CACHE[key]

    res = run_bass_kernel_spmd(nc, in_maps, core_ids=list(range(NCORES)))

    nh, NW, TH, TT = _cfg(pair_split)
    outp = np.zeros((N, R, OUT), np.float32)
    for c in range(NCORES):
        r, half = c // 2, c % 2
        o = res.results[c]["out"]
        if pair_split:
            outp[half * nh:(half + 1) * nh, r] = o[:nh]
        elif half == 0:
            outp[:, r] = o[:N]
    return outp


# revision 35
# speedup vs baseline: 1.5859x; 1.1481x over previous
"""Trainium2 Bass kernel for nn_GATv2GCN22 (4-relation GATv2 x2 + GraphConv x2).

Sharding: 8 cores; core c handles relation c//2, destination-node half c%2.
Within a relation pair, halves exchange node features between layers via
pair AllGather collectives.

Per GAT layer on each core:
  dense:  xl = h @ Wl, xr = h @ Wr over the full padded node table (PE)
  edge :  per 128-dst-node window (uniform T chunks of 128 sorted-by-dst
          edges): dma_gather xl[src] and xr[dst]; z = leaky(G + XR);
          e = reduce(z * att); p = exp(e); one-hot Se from dst-locals
          (Pool engine); rhs = [G * p | p] (264 cols);
          agg[n, 0:256|256:260] += Se_chunk^T @ rhs_chunk (PE, node-major,
          numerator and softmax denominator in one accumulator);
          h = relu(agg[:, 0:256] / agg[:, 256:260] + b) written node-major.
GraphConv layers reuse the same windows with self-loops masked out of the
one-hot, plus a fused dense epilogue per window.

Timing support: _build_nc(dyn_rep=True) wraps the network in a For_i whose
trip count is read at runtime from the `krep` input tensor, so one NEFF
serves every repeat count (collectives are skipped in that build).
"""
import numpy as np
import concourse.bacc as bacc
import concourse.tile as tile
import concourse.mybir as mybir
import concourse.bass as bass
from concourse.bass import ds, RegisterHandles, make_scalar_value
from concourse.bass_utils import run_bass_kernel_spmd

F32 = mybir.dt.float32
BF16 = mybir.dt.bfloat16
I16 = mybir.dt.int16
I32 = mybir.dt.int32
AF = mybir.ActivationFunctionType
OP = mybir.AluOpType
AX = mybir.AxisListType

N = 20000
E = 320000
R = 4
H = 4
HID = 64
DIN = 256
OUT = 64
NEG = 0.2
NCORES = 8
P = 128

_CACHE = {}


def _cfg(pair_split):
    nh = N // 2 if pair_split else N          # real nodes handled per core
    nw = -(-((nh + P - 1) // P) // 4) * 4     # 128-node windows, 4-aligned
    th = nw * P                               # padded half-table height
    tt = 2 * th if pair_split else th         # full gather-table height
    return nh, nw, th, tt


def _build_nc(T, pair_split, dyn_rep=False):
    import os
    sp = os.environ.get("KSP", "0") == "1"
    nq = int(os.environ.get("KNQ", "2"))
    scr = int(os.environ.get("KSCRATCH", "65536"))
    nh, NW, TH, TT = _cfg(pair_split)
    nc = bacc.Bacc("TRN2", target_bir_lowering=False, debug=False,
                   num_devices=NCORES, dynamic_dma_scratch_size=scr,
                   num_swdge_queues=nq)

    def inp(name, shape, dt=F32):
        return nc.dram_tensor(name, shape, dt, kind="ExternalInput").ap()

    # graph structure (shared by all 4 layers; rows stride P per window)
    src_rows = inp("src_rows", [NW * P, 8 * T], I16)
    dstc12 = inp("dstc12", [NW * P, T], BF16)
    dstc34 = inp("dstc34", [NW * P, T], BF16)
    dstr12 = inp("dstr12", [NW, T * P], BF16)
    xT = inp("xT", [P, TT // P, 2, P], BF16)
    Wl1 = inp("Wl1", [2, P, DIN], BF16)
    Wr1 = inp("Wr1", [2, P, DIN], BF16)
    Wl2 = inp("Wl2", [2, P, DIN], BF16)
    Wr2 = inp("Wr2", [2, P, DIN], BF16)
    att1 = inp("att1", [P, 1, H, HID], BF16)
    att2 = inp("att2", [P, 1, H, HID], BF16)
    b1r = inp("b1r", [P, DIN])
    b2r = inp("b2r", [P, DIN])
    Wrel3 = inp("Wrel3", [2, P, HID], BF16)
    Wroot3 = inp("Wroot3", [2, P, HID], BF16)
    Wrel4 = inp("Wrel4", [HID, OUT], BF16)
    Wroot4 = inp("Wroot4", [HID, OUT], BF16)
    b3r = inp("b3r", [P, HID])
    b4r = inp("b4r", [P, OUT])
    iota = inp("iota", [P, 1, P], BF16)
    iotac = inp("iotac", [P, 1], BF16)
    ident = inp("ident", [P, P])
    identb = inp("identb", [P, P], BF16)
    krep = inp("krep", [1, 1], I32) if dyn_rep else None
    out = nc.dram_tensor("out", [TH, OUT], F32, kind="ExternalOutput").ap()

    groups = [[0, 1], [2, 3], [4, 5], [6, 7]]

    with tile.TileContext(nc) as tc:
        with tc.tile_pool(name="dram", bufs=1, space="DRAM") as dram:
            t_xl = dram.tile([TT, DIN], BF16, name="t_xl")
            t_xr = dram.tile([TT, DIN], BF16, name="t_xr")
            t_b1 = dram.tile([TH, DIN], BF16, name="t_b1")
            t_h1 = (dram.tile([TT, DIN], BF16, name="t_h1")
                    if pair_split else t_b1)
            t_b2 = dram.tile([TH, DIN], BF16, name="t_b2")
            t_h2 = (dram.tile([TT, DIN], BF16, name="t_h2")
                    if pair_split else t_b2)
            t_b3 = dram.tile([TH, HID], F32, name="t_b3")
            t_h3 = (dram.tile([TT, HID], F32, name="t_h3")
                    if pair_split else t_b3)

            # ---------- constants resident in SBUF ----------
            with tc.tile_pool(name="const", bufs=1) as cpool:
                def const2(name, src, shape, dt=F32):
                    # src [2, P, X] -> tile [P, 2, X]
                    t = cpool.tile(shape, dt, tag=name)
                    for k in range(2):
                        nc.sync.dma_start(t[:, k], src[k])
                    return t

                def const1(name, src, shape, dt=F32):
                    t = cpool.tile(shape, dt, tag=name)
                    nc.sync.dma_start(t[:], src)
                    return t

                iota_t = const1("iota", iota[:], [P, 1, P], BF16)
                iotac_t = const1("iotac", iotac[:], [P, 1], BF16)
                id_t = const1("ident", ident[:], [P, P])
                idb_t = const1("identb", identb[:], [P, P], BF16)
                att1_t = const1("att1", att1[:], [P, 1, H, HID], BF16)
                att2_t = const1("att2", att2[:], [P, 1, H, HID], BF16)
                b1_t = const1("b1", b1r[:], [P, DIN])
                b2_t = const1("b2", b2r[:], [P, DIN])
                b3_t = const1("b3", b3r[:], [P, HID])
                b4_t = const1("b4", b4r[:], [P, OUT])
                w3l_t = const2("w3l", Wrel3, [P, 2, HID], BF16)
                w3r_t = const2("w3r", Wroot3, [P, 2, HID], BF16)
                w4l_t = const1("w4l", Wrel4[:], [HID, OUT], BF16)
                w4r_t = const1("w4r", Wroot4[:], [HID, OUT], BF16)

                # ================= phases =================

                def dense(src_h, Wl_ap, Wr_ap):
                    """xl/xr tables for all TT rows; 512-node blocks."""
                    with (
                        tc.tile_pool(name="dsb", bufs=3) as sb,
                        tc.tile_pool(name="dps", bufs=2, space="PSUM") as ps,
                        tc.tile_pool(name="dwp", bufs=1) as wp,
                    ):
                        wl_t = wp.tile([P, 2, DIN], BF16, tag="wl")
                        wr_t = wp.tile([P, 2, DIN], BF16, tag="wr")
                        for k in range(2):
                            nc.sync.dma_start(wl_t[:, k], Wl_ap[k])
                            nc.sync.dma_start(wr_t[:, k], Wr_ap[k])

                        def body(iv):
                            for s in range(4):
                                off = ds(iv + s * P, P)
                                lh = sb.tile([P, 2, P], BF16, tag="lh")
                                if src_h is None:
                                    nc.sync.dma_start(
                                        lh[:], xT[:, (iv // P) + s])
                                else:
                                    hn = sb.tile([P, DIN], BF16, tag="hn")
                                    nc.sync.dma_start(hn[:], src_h[off, :])
                                    lhp = ps.tile([P, 2, P], BF16, tag="lhp")
                                    for k in range(2):
                                        nc.tensor.transpose(
                                            lhp[:, k], hn[:, ds(k * P, P)],
                                            idb_t[:])
                                    for k in range(2):
                                        nc.vector.tensor_copy(lh[:, k],
                                                              lhp[:, k])
                                xlp = ps.tile([P, DIN], F32, tag="xlp")
                                xrp = ps.tile([P, DIN], F32, tag="xrp")
                                for k in range(2):
                                    nc.tensor.matmul(
                                        xlp[:], lh[:, k], wl_t[:, k],
                                        start=(k == 0), stop=(k == 1))
                                for k in range(2):
                                    nc.tensor.matmul(
                                        xrp[:], lh[:, k], wr_t[:, k],
                                        start=(k == 0), stop=(k == 1))
                                xls = sb.tile([P, DIN], BF16, tag="xls")
                                nc.vector.tensor_copy(xls[:], xlp[:])
                                xrs = sb.tile([P, DIN], BF16, tag="xrs")
                                nc.scalar.copy(xrs[:], xrp[:])
                                nc.sync.dma_start(t_xl[off, :], xls[:])
                                nc.sync.dma_start(t_xr[off, :], xrs[:])

                        tc.For_i_unrolled(0, TT, 4 * P, body, max_unroll=4)

                def gat_edge(att_t, b_t, t_dst, xr_off):
                    import os
                    kedge = int(os.environ.get("KEDGE", "10"))
                    with (
                        tc.tile_pool(name="esb", bufs=3) as sb,
                        tc.tile_pool(name="esm", bufs=4) as sm,
                        tc.tile_pool(name="emd", bufs=2) as md,
                        tc.tile_pool(name="eps", bufs=3, space="PSUM") as ps,
                        tc.tile_pool(name="eps2", bufs=3, space="PSUM") as ps2,
                    ):
                        def body(iv, lane=0):
                            rows = ds(iv, P)
                            isx = sm.tile([P, 8 * T], I16, tag="isx")
                            nc.sync.dma_start(isx[:], src_rows[rows, :])
                            dstc = sm.tile([P, T, 1], BF16, tag="dstc")
                            nc.sync.dma_start(dstc[:, :, 0], dstc12[rows, :])
                            dr = md.tile([1, T * P], BF16, tag="dr")
                            nc.sync.dma_start(dr[:], dstr12[ds(iv // P, 1), :])
                            xrw = sm.tile([P, DIN], BF16, tag="xrw")
                            nc.sync.dma_start(xrw[:],
                                              t_xr[ds(xr_off + iv, P), :])

                            G = sb.tile([P, T, DIN], BF16, tag="G")
                            nc.gpsimd.dma_gather(
                                out_ap=G[:], in_ap=t_xl[:, :],
                                idxs_ap=isx[:],
                                num_idxs=T * P, num_idxs_reg=T * P,
                                elem_size=DIN,
                                single_packet=sp, queue_num=lane % nq)
                            if kedge < 2:
                                nc.sync.dma_start(t_dst[rows, :], G[:, 0, :])
                                return
                            # SeT[d, e] = (d == dst_local[e]) via
                            # partition-broadcast of the dst-local row
                            db = md.tile([P, T * P], BF16, tag="db")
                            nc.gpsimd.partition_broadcast(db[:], dr[:])
                            seT = sb.tile([P, T * P], BF16, tag="seT")
                            nc.vector.tensor_tensor(
                                out=seT[:], in0=db[:],
                                in1=iotac_t[:].broadcast_to([P, T * P]),
                                op=OP.is_equal)
                            # one-hot Se[e, n] = (dstc[e] == n)
                            se = sb.tile([P, T, P], BF16, tag="se")
                            nc.vector.tensor_tensor(
                                out=se[:],
                                in0=dstc[:].broadcast_to([P, T, P]),
                                in1=iota_t[:].broadcast_to([P, T, P]),
                                op=OP.is_equal)
                            if kedge < 3:
                                nc.sync.dma_start(t_dst[rows, :],
                                                  xrw[:])
                                return
                            # z = leaky(G + xr[dst]); xr[dst] expanded from
                            # the window's 128 xr rows chunk-wise on PE
                            XR = sb.tile([P, T, DIN], BF16, tag="XR")
                            for j in range(T):
                                xrp = ps2.tile([P, DIN], F32, tag="xrp")
                                nc.tensor.matmul(
                                    xrp[:], seT[:, ds(j * P, P)], xrw[:],
                                    start=True, stop=True)
                                nc.vector.tensor_add(XR[:, j], G[:, j],
                                                     xrp[:])
                            nc.vector.scalar_tensor_tensor(
                                out=XR[:], in0=XR[:], scalar=NEG, in1=XR[:],
                                op0=OP.mult, op1=OP.max)
                            if kedge < 4:
                                nc.sync.dma_start(t_dst[rows, :], XR[:, 0, :])
                                return
                            # e = reduce(z * att); p = exp(e)
                            z4 = XR[:].rearrange("p t (h c) -> p t h c", h=H)
                            nc.vector.tensor_tensor(
                                out=z4, in0=z4,
                                in1=att_t[:].broadcast_to([P, T, H, HID]),
                                op=OP.mult)
                            pf = sm.tile([P, T, H, 1], F32, tag="pf")
                            nc.vector.tensor_reduce(
                                out=pf[:, :, :, 0], in_=z4, axis=AX.X,
                                op=OP.add)
                            nc.scalar.activation(pf[:], pf[:], AF.Exp)
                            if kedge < 5:
                                nc.sync.dma_start(t_dst[rows, 0:T],
                                                  pf[:, :, 0, 0])
                                return
                            # rhs = [G * p | p]  (264-wide, bf16)
                            gwp = sb.tile([P, T, 264], BF16, tag="gwp")
                            nc.vector.tensor_tensor(
                                out=gwp[:, :, 0:DIN].rearrange(
                                    "p t (h c) -> p t h c", h=H),
                                in0=G[:].rearrange("p t (h c) -> p t h c",
                                                   h=H),
                                in1=pf[:].broadcast_to([P, T, H, HID]),
                                op=OP.mult)
                            nc.vector.tensor_copy(gwp[:, :, DIN:DIN + H],
                                                  pf[:, :, :, 0])
                            if kedge < 6:
                                nc.sync.dma_start(t_dst[rows, :],
                                                  gwp[:, 0, 0:DIN])
                                return
                            # agg[n, 0:256] = sum_e p*G ; agg[n, 256:260] = s
                            agg = ps.tile([P, DIN + H], F32, tag="agg")
                            for j in range(T):
                                nc.tensor.matmul(
                                    agg[:], se[:, j], gwp[:, j, 0:DIN + H],
                                    start=(j == 0), stop=(j == T - 1))
                            if kedge < 7:
                                tmp7 = sm.tile([P, P], F32, tag="tmp7")
                                nc.vector.tensor_copy(tmp7[:], agg[:, 0:P])
                                nc.sync.dma_start(t_dst[rows, 0:P], tmp7[:])
                                return
                            # h = relu(agg / s + b), node-major
                            srec = sm.tile([P, H, 1], F32, tag="srec")
                            nc.vector.tensor_scalar(
                                out=srec[:, :, 0], in0=agg[:, DIN:DIN + H],
                                scalar1=1e-30, scalar2=None, op0=OP.add)
                            nc.vector.reciprocal(srec[:], srec[:])
                            hsc = sm.tile([P, H, HID], F32, tag="hsc")
                            nc.vector.tensor_tensor(
                                out=hsc[:],
                                in0=agg[:, 0:DIN].rearrange(
                                    "p (h c) -> p h c", h=H),
                                in1=srec[:].broadcast_to([P, H, HID]),
                                op=OP.mult)
                            nc.vector.tensor_add(
                                hsc[:].rearrange("p h c -> p (h c)"),
                                hsc[:].rearrange("p h c -> p (h c)"), b_t[:])
                            hb = sm.tile([P, DIN], BF16, tag="hb")
                            nc.scalar.activation(
                                hb[:], hsc[:].rearrange("p h c -> p (h c)"),
                                AF.Relu)
                            nc.sync.dma_start(t_dst[rows, :], hb[:])

                        tc.For_i_unrolled_general(
                            0, NW * P, P,
                            lambda iv0, unroll: [body(iv0 + i * P, lane=i)
                                                 for i in range(unroll)],
                            max_unroll=8)

                def gconv(t_gsrc, t_hown, wl_sl, wr_sl, b_t, t_dst, hid_out,
                          src_din, last):
                    """agg = sum h[src]; out = relu?(agg@Wl + h@Wr + b)."""
                    gdt = BF16 if src_din == DIN else F32
                    kch = max(src_din // P, 1)
                    mpart = P if kch > 1 else src_din
                    idt = idb_t if gdt == BF16 else id_t
                    with (
                        tc.tile_pool(name="gsb", bufs=3) as sb,
                        tc.tile_pool(name="gps", bufs=2, space="PSUM") as ps,
                        tc.tile_pool(name="gps1", bufs=2, space="PSUM") as ps1,
                    ):
                        def body(iv, lane=0):
                            rows = ds(iv, P)
                            isx = sb.tile([P, 8 * T], I16, tag="isx")
                            nc.sync.dma_start(isx[:], src_rows[rows, :])
                            dstc = sb.tile([P, T, 1], BF16, tag="dstc")
                            nc.sync.dma_start(dstc[:, :, 0], dstc34[rows, :])
                            G = sb.tile([P, T, src_din], gdt, tag="G")
                            nc.gpsimd.dma_gather(
                                out_ap=G[:], in_ap=t_gsrc[:, :],
                                idxs_ap=isx[:], num_idxs=T * P,
                                num_idxs_reg=T * P, elem_size=src_din,
                                single_packet=sp, queue_num=lane % nq)
                            se = sb.tile([P, T, P], BF16, tag="se")
                            nc.vector.tensor_tensor(
                                out=se[:],
                                in0=dstc[:].broadcast_to([P, T, P]),
                                in1=iota_t[:].broadcast_to([P, T, P]),
                                op=OP.is_equal)
                            if gdt == BF16:
                                gb = G
                            else:
                                gb = sb.tile([P, T, src_din], BF16, tag="gb")
                                nc.scalar.copy(gb[:], G[:])
                            agg = ps.tile([mpart, kch, P], F32, tag="agg")
                            for k in range(kch):
                                for j in range(T):
                                    nc.tensor.matmul(
                                        agg[:, k],
                                        gb[:, j, ds(k * P, P)] if kch > 1
                                        else gb[:, j],
                                        se[:, j], start=(j == 0),
                                        stop=(j == T - 1))
                            # fused dense epilogue
                            hw = sb.tile([P, src_din], gdt, tag="hw")
                            nc.sync.dma_start(hw[:], t_hown[rows, :])
                            hTp = ps1.tile([mpart, kch, P], gdt, tag="hTp")
                            for k in range(kch):
                                nc.tensor.transpose(
                                    hTp[:, k],
                                    hw[:, ds(k * P, P)] if kch > 1 else hw[:],
                                    idt[:])
                            aT = sb.tile([mpart, kch, P], BF16, tag="aT")
                            nc.vector.tensor_copy(aT[:], agg[:])
                            hT = sb.tile([mpart, kch, P], BF16, tag="hTt")
                            nc.vector.tensor_copy(hT[:], hTp[:])
                            op_ = ps.tile([P, hid_out], F32, tag="op")
                            for k in range(kch):
                                nc.tensor.matmul(op_[:], aT[:, k], wl_sl[k],
                                                 start=(k == 0), stop=False)
                            for k in range(kch):
                                nc.tensor.matmul(op_[:], hT[:, k], wr_sl[k],
                                                 start=False,
                                                 stop=(k == kch - 1))
                            os_ = sb.tile([P, hid_out], F32, tag="os")
                            nc.vector.tensor_add(os_[:], op_[:], b_t[:])
                            if not last:
                                nc.vector.tensor_scalar_max(os_[:], os_[:],
                                                            0.0)
                            nc.sync.dma_start(t_dst[rows, :], os_[:])

                        tc.For_i_unrolled_general(
                            0, NW * P, P,
                            lambda iv0, unroll: [body(iv0 + i * P, lane=i)
                                                 for i in range(unroll)],
                            max_unroll=8)

                def exchange(src_t, dst_t):
                    import os
                    if (not pair_split or dyn_rep
                            or os.environ.get("KNOCC") == "1"):
                        return
                    nc.gpsimd.collective_compute(
                        "AllGather", OP.bypass, replica_groups=groups,
                        ins=[src_t.opt()], outs=[dst_t.opt()])

                # ================= the network =================
                import os
                phases = int(os.environ.get("KPHASES", "7"))
                pid = nc.sync.partition_id()
                xr_off = nc.s_assert_within((pid % 2) * TH, 0, TH)

                def network():
                    if phases >= 1:
                        dense(None, Wl1, Wr1)
                    if phases >= 2:
                        gat_edge(att1_t, b1_t, t_b1, xr_off)
                    if phases >= 3:
                        exchange(t_b1, t_h1)
                    if phases >= 4:
                        dense(t_h1, Wl2, Wr2)
                    if phases >= 5:
                        gat_edge(att2_t, b2_t, t_b2, xr_off)
                        exchange(t_b2, t_h2)
                    if phases >= 6:
                        gconv(t_h2, t_b2, [w3l_t[:, 0], w3l_t[:, 1]],
                              [w3r_t[:, 0], w3r_t[:, 1]], b3_t, t_b3, HID,
                              DIN, False)
                    if phases >= 7:
                        exchange(t_b3, t_h3)
                        gconv(t_h3, t_b3, [w4l_t[:]], [w4r_t[:]], b4_t, out,
                              OUT, HID, True)

                if dyn_rep:
                    regs = []
                    for e in mybir.ALL_ENGINES:
                        eng = nc.engines[e]
                        r = eng.alloc_register(f"krep_{e.name}")
                        eng.reg_load(r, krep[0:1, 0:1])
                        regs.append(r)
                    kval = make_scalar_value(RegisterHandles(regs),
                                             min_val=0, max_val=1 << 20)
                    with tc.For_i(0, kval, 1):
                        network()
                else:
                    krep_env = int(os.environ.get("KREP", "1"))
                    if krep_env > 1:
                        with tc.For_i(0, krep_env, 1):
                            network()
                    else:
                        network()

    nc.compile()
    return nc


def _host_prep(x, edge_indices, Wl1, Wr1, att1, b1, Wl2, Wr2, att2, b2,
               Wrel3, Wroot3, b3, Wrel4, Wroot4, b4, pair_split=True):
    import ml_dtypes
    nh, NW, TH, TT = _cfg(pair_split)
    x = np.asarray(x, np.float32)
    ei = np.asarray(edge_indices)

    def glob_row(n):
        if pair_split:
            return np.where(n < nh, n, TH + (n - nh))
        return n

    structs = []
    chunk_counts = []
    for c in range(NCORES):
        r, half = c // 2, c % 2
        src, dst = ei[r, 0].astype(np.int64), ei[r, 1].astype(np.int64)
        if pair_split:
            m = (dst >= half * nh) & (dst < (half + 1) * nh)
            src, dst = src[m], dst[m] - half * nh
        loops_src = np.arange(nh) + (half * nh if pair_split else 0)
        s_all = np.concatenate([src, loops_src])
        d_all = np.concatenate([dst, np.arange(nh)])
        isloop = np.zeros(len(s_all), bool)
        isloop[len(src):] = True
        order = np.argsort(d_all, kind="stable")
        s_all, d_all, isloop = s_all[order], d_all[order], isloop[order]
        counts = np.bincount(d_all // P, minlength=NW)
        chunk_counts.append(np.ceil(counts / P).astype(int))
        structs.append((s_all, d_all, isloop, counts))

    T = int(max(cc.max() for cc in chunk_counts))

    # shared constants
    xpad = np.zeros((TT, DIN), np.float32)
    xpad[:nh] = x[:nh]
    if pair_split:
        xpad[TH:TH + nh] = x[nh:]
    # [P, TT//P, 2, P]: one 512B read per 128-node block in dense()
    xT = np.ascontiguousarray(
        xpad.reshape(TT // P, P, 2, P).transpose(3, 0, 2, 1))\
        .astype(ml_dtypes.bfloat16)
    iota_c = np.broadcast_to(
        np.arange(P, dtype=ml_dtypes.bfloat16)[None, None], (P, 1, P)).copy()
    iotac_c = np.arange(P, dtype=ml_dtypes.bfloat16).reshape(P, 1).copy()
    ident_c = np.eye(P, dtype=np.float32)
    identb_c = np.eye(P, dtype=ml_dtypes.bfloat16)

    in_maps = []
    for c in range(NCORES):
        r, half = c // 2, c % 2
        s_all, d_all, isloop, counts = structs[c]
        srcr = np.zeros((NW, T * P), np.int64)
        dstr = np.zeros((NW, T * P), np.int64)
        dc12 = np.full((NW, T * P), -1.0, np.float32)
        dc34 = np.full((NW, T * P), -1.0, np.float32)
        start = 0
        off = half * TH if pair_split else 0
        for w in range(NW):
            cnt = counts[w]
            sl = slice(start, start + cnt)
            start += cnt
            srcr[w, :cnt] = glob_row(s_all[sl])
            dstr[w, :cnt] = off + d_all[sl]
            dl = (d_all[sl] - w * P).astype(np.float32)
            dc12[w, :cnt] = dl
            dc34[w, :cnt] = np.where(isloop[sl], -1.0, dl)

        def wrap(a):  # [NW, T*P] -> [NW*P, 8T] int16, replicated per Q7 core
            b = a.astype(np.int16).reshape(NW, 8 * T, 16)
            b = np.transpose(b, (0, 2, 1))
            return np.tile(b, (1, 8, 1)).reshape(NW * P, 8 * T).copy()

        def colmajor(a):  # [NW, T*P] -> [NW*P, T]
            return np.ascontiguousarray(
                np.transpose(a.reshape(NW, T, P), (0, 2, 1))).reshape(
                    NW * P, T)

        kchunk = lambda w: np.ascontiguousarray(
            np.asarray(w[r], np.float32)).reshape(2, P, -1)
        att_rep = lambda a: np.broadcast_to(
            np.asarray(a[r], np.float32).astype(ml_dtypes.bfloat16)
            [None, None], (P, 1, H, HID)).copy()
        brep = lambda b, n: np.broadcast_to(
            np.asarray(b[r], np.float32)[None], (P, n)).copy()
        bf = lambda a: np.ascontiguousarray(a).astype(ml_dtypes.bfloat16)

        in_maps.append(dict(
            src_rows=wrap(srcr),
            dstc12=colmajor(dc12).astype(ml_dtypes.bfloat16),
            dstc34=colmajor(dc34).astype(ml_dtypes.bfloat16),
            dstr12=dc12.astype(ml_dtypes.bfloat16),
            xT=xT,
            Wl1=bf(kchunk(Wl1)), Wr1=bf(kchunk(Wr1)),
            Wl2=bf(kchunk(Wl2)), Wr2=bf(kchunk(Wr2)),
            att1=att_rep(att1), att2=att_rep(att2),
            b1r=brep(b1, DIN), b2r=brep(b2, DIN),
            Wrel3=bf(kchunk(Wrel3)), Wroot3=bf(kchunk(Wroot3)),
            Wrel4=bf(np.asarray(Wrel4[r], np.float32)),
            Wroot4=bf(np.asarray(Wroot4[r], np.float32)),
            b3r=brep(b3, HID), b4r=brep(b4, OUT),
            iota=iota_c, iotac=iotac_c, ident=ident_c, identb=identb_c,
        ))
    return in_maps, T


def kernel(x, edge_indices, Wl1, Wr1, att1, b1, Wl2, Wr2, att2, b2,
           Wrel3, Wroot3, b3, Wrel4, Wroot4, b4, pair_split=True):
    in_maps, T = _host_prep(x, edge_indices, Wl1, Wr1, att1, b1, Wl2, Wr2,
                            att2, b2, Wrel3, Wroot3, b3, Wrel4, Wroot4, b4,
                            pair_split)
    import os
    key = (T, pair_split, os.environ.get("KPHASES"), os.environ.get("KEDGE"),
           os.environ.get("KNOCC"), os.environ.get("KREP"))
    if key not in _CACHE:
        _CACHE[key] = _build_nc(T, pair_split)
    nc = _CACHE[key]

    res = run_bass_kernel_spmd(nc, in_maps, core_ids=list(range(NCORES)))

    nh, NW, TH, TT = _cfg(pair_split)
    outp = np.zeros((N, R, OUT), np.float32)
    for c in range(NCORES):
        r, half = c // 2, c % 2
        o = res.results[c]["out"]
        if pair_split:
            outp[half * nh:(half + 1) * nh, r] = o[:nh]
        elif half == 0:
            outp[:, r] = o[:N]
    return outp


# revision 42
# speedup vs baseline: 1.6337x; 1.0302x over previous
"""Trainium2 Bass kernel for nn_GATv2GCN22 (4-relation GATv2 x2 + GraphConv x2).

Sharding: 8 cores; core c handles relation c//2, destination-node half c%2.
Within a relation pair, halves exchange node features between layers via
pair AllGather collectives.

Per GAT layer on each core:
  dense:  xl = h @ Wl, xr = h @ Wr over the full padded node table (PE)
  edge :  per 128-dst-node window (uniform T chunks of 128 sorted-by-dst
          edges): dma_gather xl[src] and xr[dst]; z = leaky(G + XR);
          e = reduce(z * att); p = exp(e); one-hot Se from dst-locals
          (Pool engine); rhs = [G * p | p] (264 cols);
          agg[n, 0:256|256:260] += Se_chunk^T @ rhs_chunk (PE, node-major,
          numerator and softmax denominator in one accumulator);
          h = relu(agg[:, 0:256] / agg[:, 256:260] + b) written node-major.
GraphConv layers reuse the same windows with self-loops masked out of the
one-hot, plus a fused dense epilogue per window.

Timing support: _build_nc(dyn_rep=True) wraps the network in a For_i whose
trip count is read at runtime from the `krep` input tensor, so one NEFF
serves every repeat count (collectives are skipped in that build).
"""
import numpy as np
import concourse.bacc as bacc
import concourse.tile as tile
import concourse.mybir as mybir
import concourse.bass as bass
from concourse.bass import ds, RegisterHandles, make_scalar_value
from concourse.bass_utils import run_bass_kernel_spmd

F32 = mybir.dt.float32
BF16 = mybir.dt.bfloat16
I16 = mybir.dt.int16
I32 = mybir.dt.int32
AF = mybir.ActivationFunctionType
OP = mybir.AluOpType
AX = mybir.AxisListType

N = 20000
E = 320000
R = 4
H = 4
HID = 64
DIN = 256
OUT = 64
NEG = 0.2
NCORES = 8
P = 128

_CACHE = {}


def _cfg(pair_split):
    nh = N // 2 if pair_split else N          # real nodes handled per core
    nw = -(-((nh + P - 1) // P) // 4) * 4     # 128-node windows, 4-aligned
    th = nw * P                               # padded half-table height
    tt = 2 * th if pair_split else th         # full gather-table height
    return nh, nw, th, tt


def _build_nc(T, pair_split, dyn_rep=False):
    import os
    sp = os.environ.get("KSP", "0") == "1"
    nq = int(os.environ.get("KNQ", "2"))
    scr = int(os.environ.get("KSCRATCH", "65536"))
    nh, NW, TH, TT = _cfg(pair_split)
    nc = bacc.Bacc("TRN2", target_bir_lowering=False, debug=False,
                   num_devices=NCORES, dynamic_dma_scratch_size=scr,
                   num_swdge_queues=nq)

    def inp(name, shape, dt=F32):
        return nc.dram_tensor(name, shape, dt, kind="ExternalInput").ap()

    # graph structure (shared by all 4 layers; rows stride P per window)
    src_rows = inp("src_rows", [NW * P, 8 * T], I16)
    src_off = inp("src_off", [NW * P, T], I32)
    dstc12 = inp("dstc12", [NW * P, T], BF16)
    dstc34 = inp("dstc34", [NW * P, T], BF16)
    dstr12 = inp("dstr12", [NW, T * P], BF16)
    xT = inp("xT", [P, TT // P, 2, P], BF16)
    Wl1 = inp("Wl1", [2, P, DIN], BF16)
    Wr1 = inp("Wr1", [2, P, DIN], BF16)
    Wl2 = inp("Wl2", [2, P, DIN], BF16)
    Wr2 = inp("Wr2", [2, P, DIN], BF16)
    att1 = inp("att1", [P, 1, H, HID], BF16)
    att2 = inp("att2", [P, 1, H, HID], BF16)
    b1r = inp("b1r", [P, DIN])
    b2r = inp("b2r", [P, DIN])
    Wrel3 = inp("Wrel3", [2, P, HID], BF16)
    Wroot3 = inp("Wroot3", [2, P, HID], BF16)
    Wrel4 = inp("Wrel4", [HID, OUT], BF16)
    Wroot4 = inp("Wroot4", [HID, OUT], BF16)
    b3r = inp("b3r", [P, HID])
    b4r = inp("b4r", [P, OUT])
    iota = inp("iota", [P, 1, P], BF16)
    iotac = inp("iotac", [P, 1], BF16)
    ident = inp("ident", [P, P])
    identb = inp("identb", [P, P], BF16)
    krep = inp("krep", [1, 1], I32) if dyn_rep else None
    out = nc.dram_tensor("out", [TH, OUT], F32, kind="ExternalOutput").ap()

    groups = [[0, 1], [2, 3], [4, 5], [6, 7]]

    with tile.TileContext(nc) as tc:
        with tc.tile_pool(name="dram", bufs=1, space="DRAM") as dram:
            t_xl = dram.tile([TT, DIN], BF16, name="t_xl")
            t_xr = dram.tile([TT, DIN], BF16, name="t_xr")
            t_b1 = dram.tile([TH, DIN], BF16, name="t_b1")
            t_h1 = (dram.tile([TT, DIN], BF16, name="t_h1")
                    if pair_split else t_b1)
            t_b2 = dram.tile([TH, DIN], BF16, name="t_b2")
            t_h2 = (dram.tile([TT, DIN], BF16, name="t_h2")
                    if pair_split else t_b2)
            t_b3 = dram.tile([TH, HID], F32, name="t_b3")
            t_h3 = (dram.tile([TT, HID], F32, name="t_h3")
                    if pair_split else t_b3)

            # ---------- constants resident in SBUF ----------
            with tc.tile_pool(name="const", bufs=1) as cpool:
                def const2(name, src, shape, dt=F32):
                    # src [2, P, X] -> tile [P, 2, X]
                    t = cpool.tile(shape, dt, tag=name)
                    for k in range(2):
                        nc.sync.dma_start(t[:, k], src[k])
                    return t

                def const1(name, src, shape, dt=F32):
                    t = cpool.tile(shape, dt, tag=name)
                    nc.sync.dma_start(t[:], src)
                    return t

                iota_t = const1("iota", iota[:], [P, 1, P], BF16)
                iotac_t = const1("iotac", iotac[:], [P, 1], BF16)
                id_t = const1("ident", ident[:], [P, P])
                idb_t = const1("identb", identb[:], [P, P], BF16)
                att1_t = const1("att1", att1[:], [P, 1, H, HID], BF16)
                att2_t = const1("att2", att2[:], [P, 1, H, HID], BF16)
                b1_t = const1("b1", b1r[:], [P, DIN])
                b2_t = const1("b2", b2r[:], [P, DIN])
                b3_t = const1("b3", b3r[:], [P, HID])
                b4_t = const1("b4", b4r[:], [P, OUT])
                w3l_t = const2("w3l", Wrel3, [P, 2, HID], BF16)
                w3r_t = const2("w3r", Wroot3, [P, 2, HID], BF16)
                w4l_t = const1("w4l", Wrel4[:], [HID, OUT], BF16)
                w4r_t = const1("w4r", Wroot4[:], [HID, OUT], BF16)

                # ================= phases =================

                def dense(src_h, Wl_ap, Wr_ap):
                    """xl/xr tables for all TT rows; 512-node blocks."""
                    with (
                        tc.tile_pool(name="dsb", bufs=3) as sb,
                        tc.tile_pool(name="dps", bufs=2, space="PSUM") as ps,
                        tc.tile_pool(name="dwp", bufs=1) as wp,
                    ):
                        wl_t = wp.tile([P, 2, DIN], BF16, tag="wl")
                        wr_t = wp.tile([P, 2, DIN], BF16, tag="wr")
                        for k in range(2):
                            nc.sync.dma_start(wl_t[:, k], Wl_ap[k])
                            nc.sync.dma_start(wr_t[:, k], Wr_ap[k])

                        def body(iv):
                            for s in range(4):
                                off = ds(iv + s * P, P)
                                lh = sb.tile([P, 2, P], BF16, tag="lh")
                                if src_h is None:
                                    nc.sync.dma_start(
                                        lh[:], xT[:, (iv // P) + s])
                                else:
                                    hn = sb.tile([P, DIN], BF16, tag="hn")
                                    nc.sync.dma_start(hn[:], src_h[off, :])
                                    lhp = ps.tile([P, 2, P], BF16, tag="lhp")
                                    for k in range(2):
                                        nc.tensor.transpose(
                                            lhp[:, k], hn[:, ds(k * P, P)],
                                            idb_t[:])
                                    for k in range(2):
                                        nc.vector.tensor_copy(lh[:, k],
                                                              lhp[:, k])
                                xlp = ps.tile([P, DIN], F32, tag="xlp")
                                xrp = ps.tile([P, DIN], F32, tag="xrp")
                                for k in range(2):
                                    nc.tensor.matmul(
                                        xlp[:], lh[:, k], wl_t[:, k],
                                        start=(k == 0), stop=(k == 1))
                                for k in range(2):
                                    nc.tensor.matmul(
                                        xrp[:], lh[:, k], wr_t[:, k],
                                        start=(k == 0), stop=(k == 1))
                                xls = sb.tile([P, DIN], BF16, tag="xls")
                                nc.vector.tensor_copy(xls[:], xlp[:])
                                xrs = sb.tile([P, DIN], BF16, tag="xrs")
                                nc.scalar.copy(xrs[:], xrp[:])
                                nc.sync.dma_start(t_xl[off, :], xls[:])
                                nc.sync.dma_start(t_xr[off, :], xrs[:])

                        tc.For_i_unrolled(0, TT, 4 * P, body, max_unroll=4)

                def gat_edge(att_t, b_t, t_dst, xr_off):
                    import os
                    kedge = int(os.environ.get("KEDGE", "10"))
                    with (
                        tc.tile_pool(name="esb", bufs=3) as sb,
                        tc.tile_pool(name="esm", bufs=4) as sm,
                        tc.tile_pool(name="emd", bufs=3) as md,
                        tc.tile_pool(name="eps", bufs=3, space="PSUM") as ps,
                        tc.tile_pool(name="eps2", bufs=2, space="PSUM") as ps2,
                    ):
                        def body(iv, lane=0):
                            rows = ds(iv, P)
                            isx = sm.tile([P, 8 * T], I16, tag="isx")
                            nc.sync.dma_start(isx[:], src_rows[rows, :])
                            dstc = sm.tile([P, T, 1], BF16, tag="dstc")
                            nc.sync.dma_start(dstc[:, :, 0], dstc12[rows, :])
                            dr = md.tile([1, T * P], BF16, tag="dr")
                            nc.sync.dma_start(dr[:], dstr12[ds(iv // P, 1), :])
                            xrw = sm.tile([P, DIN], BF16, tag="xrw")
                            nc.sync.dma_start(xrw[:],
                                              t_xr[ds(xr_off + iv, P), :])

                            if os.environ.get("KIND") == "ind" and kedge < 2:
                                offs = sm.tile([P, T, 1], I32, tag="offs")
                                nc.sync.dma_start(offs[:, :, 0],
                                                  src_off[rows, :])
                                G = sb.tile([P, T, DIN], BF16, tag="G")
                                nc.gpsimd.indirect_dma_start(
                                    out=G[:], out_offset=None,
                                    in_=t_xl[:, :],
                                    in_offset=bass.IndirectOffsetOnAxis(
                                        ap=offs[:], axis=0))
                                nc.sync.dma_start(t_dst[rows, :], G[:, 0, :])
                                return
                            if os.environ.get("KIND") == "tr" and kedge < 2:
                                GT = sb.tile([P, 2, T * P], BF16, tag="G")
                                nc.gpsimd.dma_gather(
                                    out_ap=GT[:], in_ap=t_xl[:, :],
                                    idxs_ap=isx[:],
                                    num_idxs=T * P, num_idxs_reg=T * P,
                                    elem_size=DIN, transpose=True,
                                    single_packet=sp, queue_num=lane % nq)
                                nc.sync.dma_start(t_dst[rows, :],
                                                  GT[:, :, 0:P])
                                return
                            kni = int(os.environ.get("KNI", str(T * P)))
                            G = sb.tile([P, T, DIN], BF16, tag="G")
                            nc.gpsimd.dma_gather(
                                out_ap=G[:], in_ap=t_xl[:, :],
                                idxs_ap=isx[:],
                                num_idxs=T * P, num_idxs_reg=kni,
                                elem_size=DIN,
                                single_packet=sp, queue_num=lane % nq)
                            if kedge < 2:
                                nc.sync.dma_start(t_dst[rows, :], G[:, 0, :])
                                return
                            # SeT[d, e] = (d == dst_local[e]) via
                            # partition-broadcast of the dst-local row
                            db = md.tile([P, T * P], BF16, tag="db")
                            nc.gpsimd.partition_broadcast(db[:], dr[:])
                            seT = sb.tile([P, T * P], BF16, tag="seT")
                            nc.vector.tensor_tensor(
                                out=seT[:], in0=db[:],
                                in1=iotac_t[:].broadcast_to([P, T * P]),
                                op=OP.is_equal)
                            # one-hot Se[e, n] = (dstc[e] == n)
                            se = sb.tile([P, T, P], BF16, tag="se")
                            nc.vector.tensor_tensor(
                                out=se[:],
                                in0=dstc[:].broadcast_to([P, T, P]),
                                in1=iota_t[:].broadcast_to([P, T, P]),
                                op=OP.is_equal)
                            if kedge < 3:
                                nc.sync.dma_start(t_dst[rows, :],
                                                  xrw[:])
                                return
                            # z = leaky(G + xr[dst]); xr[dst] expanded from
                            # the window's 128 xr rows, 4 chunks per group
                            XR = sb.tile([P, T, DIN], BF16, tag="XR")
                            for g in range((T + 3) // 4):
                                nj = min(4, T - 4 * g)
                                xrp = ps2.tile([P, 4, DIN], F32, tag="xrp")
                                for jj in range(nj):
                                    j = 4 * g + jj
                                    nc.tensor.matmul(
                                        xrp[:, jj], seT[:, ds(j * P, P)],
                                        xrw[:], start=True, stop=True)
                                nc.vector.tensor_add(
                                    XR[:, ds(4 * g, nj)], G[:, ds(4 * g, nj)],
                                    xrp[:, 0:nj])
                            nc.vector.scalar_tensor_tensor(
                                out=XR[:], in0=XR[:], scalar=NEG, in1=XR[:],
                                op0=OP.mult, op1=OP.max)
                            if kedge < 4:
                                nc.sync.dma_start(t_dst[rows, :], XR[:, 0, :])
                                return
                            # e = reduce(z * att); p = exp(e)
                            z4 = XR[:].rearrange("p t (h c) -> p t h c", h=H)
                            nc.vector.tensor_tensor(
                                out=z4, in0=z4,
                                in1=att_t[:].broadcast_to([P, T, H, HID]),
                                op=OP.mult)
                            pf = sm.tile([P, T, H, 1], F32, tag="pf")
                            nc.vector.tensor_reduce(
                                out=pf[:, :, :, 0], in_=z4, axis=AX.X,
                                op=OP.add)
                            nc.scalar.activation(pf[:], pf[:], AF.Exp)
                            if kedge < 5:
                                nc.sync.dma_start(t_dst[rows, 0:T],
                                                  pf[:, :, 0, 0])
                                return
                            # rhs = [G * p | p]  (264-wide, bf16)
                            gwp = sb.tile([P, T, 264], BF16, tag="gwp")
                            nc.vector.tensor_tensor(
                                out=gwp[:, :, 0:DIN].rearrange(
                                    "p t (h c) -> p t h c", h=H),
                                in0=G[:].rearrange("p t (h c) -> p t h c",
                                                   h=H),
                                in1=pf[:].broadcast_to([P, T, H, HID]),
                                op=OP.mult)
                            nc.vector.tensor_copy(gwp[:, :, DIN:DIN + H],
                                                  pf[:, :, :, 0])
                            if kedge < 6:
                                nc.sync.dma_start(t_dst[rows, :],
                                                  gwp[:, 0, 0:DIN])
                                return
                            # agg[n, 0:256] = sum_e p*G ; agg[n, 256:260] = s
                            agg = ps.tile([P, DIN + H], F32, tag="agg")
                            for j in range(T):
                                nc.tensor.matmul(
                                    agg[:], se[:, j], gwp[:, j, 0:DIN + H],
                                    start=(j == 0), stop=(j == T - 1))
                            if kedge < 7:
                                tmp7 = sm.tile([P, P], F32, tag="tmp7")
                                nc.vector.tensor_copy(tmp7[:], agg[:, 0:P])
                                nc.sync.dma_start(t_dst[rows, 0:P], tmp7[:])
                                return
                            # h = relu(agg / s + b), node-major
                            srec = sm.tile([P, H, 1], F32, tag="srec")
                            nc.vector.tensor_scalar(
                                out=srec[:, :, 0], in0=agg[:, DIN:DIN + H],
                                scalar1=1e-30, scalar2=None, op0=OP.add)
                            nc.vector.reciprocal(srec[:], srec[:])
                            hsc = sm.tile([P, H, HID], F32, tag="hsc")
                            nc.vector.tensor_tensor(
                                out=hsc[:],
                                in0=agg[:, 0:DIN].rearrange(
                                    "p (h c) -> p h c", h=H),
                                in1=srec[:].broadcast_to([P, H, HID]),
                                op=OP.mult)
                            nc.vector.tensor_add(
                                hsc[:].rearrange("p h c -> p (h c)"),
                                hsc[:].rearrange("p h c -> p (h c)"), b_t[:])
                            hb = sm.tile([P, DIN], BF16, tag="hb")
                            nc.scalar.activation(
                                hb[:], hsc[:].rearrange("p h c -> p (h c)"),
                                AF.Relu)
                            nc.sync.dma_start(t_dst[rows, :], hb[:])

                        tc.For_i_unrolled_general(
                            0, NW * P, P,
                            lambda iv0, unroll: [body(iv0 + i * P, lane=i)
                                                 for i in range(unroll)],
                            max_unroll=8)

                def gconv(t_gsrc, t_hown, wl_sl, wr_sl, b_t, t_dst, hid_out,
                          src_din, last):
                    """agg = sum h[src]; out = relu?(agg@Wl + h@Wr + b)."""
                    gdt = BF16 if src_din == DIN else F32
                    kch = max(src_din // P, 1)
                    mpart = P if kch > 1 else src_din
                    idt = idb_t if gdt == BF16 else id_t
                    with (
                        tc.tile_pool(name="gsb", bufs=3) as sb,
                        tc.tile_pool(name="gps", bufs=2, space="PSUM") as ps,
                        tc.tile_pool(name="gps1", bufs=2, space="PSUM") as ps1,
                    ):
                        def body(iv, lane=0):
                            rows = ds(iv, P)
                            isx = sb.tile([P, 8 * T], I16, tag="isx")
                            nc.sync.dma_start(isx[:], src_rows[rows, :])
                            dstc = sb.tile([P, T, 1], BF16, tag="dstc")
                            nc.sync.dma_start(dstc[:, :, 0], dstc34[rows, :])
                            G = sb.tile([P, T, src_din], gdt, tag="G")
                            nc.gpsimd.dma_gather(
                                out_ap=G[:], in_ap=t_gsrc[:, :],
                                idxs_ap=isx[:], num_idxs=T * P,
                                num_idxs_reg=T * P, elem_size=src_din,
                                single_packet=sp, queue_num=lane % nq)
                            se = sb.tile([P, T, P], BF16, tag="se")
                            nc.vector.tensor_tensor(
                                out=se[:],
                                in0=dstc[:].broadcast_to([P, T, P]),
                                in1=iota_t[:].broadcast_to([P, T, P]),
                                op=OP.is_equal)
                            if gdt == BF16:
                                gb = G
                            else:
                                gb = sb.tile([P, T, src_din], BF16, tag="gb")
                                nc.scalar.copy(gb[:], G[:])
                            agg = ps.tile([mpart, kch, P], F32, tag="agg")
                            for k in range(kch):
                                for j in range(T):
                                    nc.tensor.matmul(
                                        agg[:, k],
                                        gb[:, j, ds(k * P, P)] if kch > 1
                                        else gb[:, j],
                                        se[:, j], start=(j == 0),
                                        stop=(j == T - 1))
                            # fused dense epilogue
                            hw = sb.tile([P, src_din], gdt, tag="hw")
                            nc.sync.dma_start(hw[:], t_hown[rows, :])
                            hTp = ps1.tile([mpart, kch, P], gdt, tag="hTp")
                            for k in range(kch):
                                nc.tensor.transpose(
                                    hTp[:, k],
                                    hw[:, ds(k * P, P)] if kch > 1 else hw[:],
                                    idt[:])
                            aT = sb.tile([mpart, kch, P], BF16, tag="aT")
                            nc.vector.tensor_copy(aT[:], agg[:])
                            hT = sb.tile([mpart, kch, P], BF16, tag="hTt")
                            nc.vector.tensor_copy(hT[:], hTp[:])
                            op_ = ps.tile([P, hid_out], F32, tag="op")
                            for k in range(kch):
                                nc.tensor.matmul(op_[:], aT[:, k], wl_sl[k],
                                                 start=(k == 0), stop=False)
                            for k in range(kch):
                                nc.tensor.matmul(op_[:], hT[:, k], wr_sl[k],
                                                 start=False,
                                                 stop=(k == kch - 1))
                            os_ = sb.tile([P, hid_out], F32, tag="os")
                            nc.vector.tensor_add(os_[:], op_[:], b_t[:])
                            if not last:
                                nc.vector.tensor_scalar_max(os_[:], os_[:],
                                                            0.0)
                            nc.sync.dma_start(t_dst[rows, :], os_[:])

                        tc.For_i_unrolled_general(
                            0, NW * P, P,
                            lambda iv0, unroll: [body(iv0 + i * P, lane=i)
                                                 for i in range(unroll)],
                            max_unroll=8)

                def exchange(src_t, dst_t):
                    import os
                    if (not pair_split or dyn_rep
                            or os.environ.get("KNOCC") == "1"):
                        return
                    nc.gpsimd.collective_compute(
                        "AllGather", OP.bypass, replica_groups=groups,
                        ins=[src_t.opt()], outs=[dst_t.opt()])

                # ================= the network =================
                import os
                phases = int(os.environ.get("KPHASES", "7"))
                pid = nc.sync.partition_id()
                xr_off = nc.s_assert_within((pid % 2) * TH, 0, TH)

                def network():
                    if phases >= 1:
                        dense(None, Wl1, Wr1)
                    if phases >= 2:
                        gat_edge(att1_t, b1_t, t_b1, xr_off)
                    if phases >= 3:
                        exchange(t_b1, t_h1)
                    if phases >= 4:
                        dense(t_h1, Wl2, Wr2)
                    if phases >= 5:
                        gat_edge(att2_t, b2_t, t_b2, xr_off)
                        exchange(t_b2, t_h2)
                    if phases >= 6:
                        gconv(t_h2, t_b2, [w3l_t[:, 0], w3l_t[:, 1]],
                              [w3r_t[:, 0], w3r_t[:, 1]], b3_t, t_b3, HID,
                              DIN, False)
                    if phases >= 7:
                        exchange(t_b3, t_h3)
                        gconv(t_h3, t_b3, [w4l_t[:]], [w4r_t[:]], b4_t, out,
                              OUT, HID, True)

                if dyn_rep:
                    regs = []
                    for e in mybir.ALL_ENGINES:
                        eng = nc.engines[e]
                        r = eng.alloc_register(f"krep_{e.name}")
                        eng.reg_load(r, krep[0:1, 0:1])
                        regs.append(r)
                    kval = make_scalar_value(RegisterHandles(regs),
                                             min_val=0, max_val=1 << 20)
                    with tc.For_i(0, kval, 1):
                        network()
                else:
                    krep_env = int(os.environ.get("KREP", "1"))
                    if krep_env > 1:
                        with tc.For_i(0, krep_env, 1):
                            network()
                    else:
                        network()

    nc.compile()
    return nc


def _host_prep(x, edge_indices, Wl1, Wr1, att1, b1, Wl2, Wr2, att2, b2,
               Wrel3, Wroot3, b3, Wrel4, Wroot4, b4, pair_split=True):
    import ml_dtypes
    nh, NW, TH, TT = _cfg(pair_split)
    x = np.asarray(x, np.float32)
    ei = np.asarray(edge_indices)

    def glob_row(n):
        if pair_split:
            return np.where(n < nh, n, TH + (n - nh))
        return n

    structs = []
    chunk_counts = []
    for c in range(NCORES):
        r, half = c // 2, c % 2
        src, dst = ei[r, 0].astype(np.int64), ei[r, 1].astype(np.int64)
        if pair_split:
            m = (dst >= half * nh) & (dst < (half + 1) * nh)
            src, dst = src[m], dst[m] - half * nh
        loops_src = np.arange(nh) + (half * nh if pair_split else 0)
        s_all = np.concatenate([src, loops_src])
        d_all = np.concatenate([dst, np.arange(nh)])
        isloop = np.zeros(len(s_all), bool)
        isloop[len(src):] = True
        order = np.argsort(d_all, kind="stable")
        s_all, d_all, isloop = s_all[order], d_all[order], isloop[order]
        counts = np.bincount(d_all // P, minlength=NW)
        chunk_counts.append(np.ceil(counts / P).astype(int))
        structs.append((s_all, d_all, isloop, counts))

    T = int(max(cc.max() for cc in chunk_counts))

    # shared constants
    xpad = np.zeros((TT, DIN), np.float32)
    xpad[:nh] = x[:nh]
    if pair_split:
        xpad[TH:TH + nh] = x[nh:]
    # [P, TT//P, 2, P]: one 512B read per 128-node block in dense()
    xT = np.ascontiguousarray(
        xpad.reshape(TT // P, P, 2, P).transpose(3, 0, 2, 1))\
        .astype(ml_dtypes.bfloat16)
    iota_c = np.broadcast_to(
        np.arange(P, dtype=ml_dtypes.bfloat16)[None, None], (P, 1, P)).copy()
    iotac_c = np.arange(P, dtype=ml_dtypes.bfloat16).reshape(P, 1).copy()
    ident_c = np.eye(P, dtype=np.float32)
    identb_c = np.eye(P, dtype=ml_dtypes.bfloat16)

    in_maps = []
    for c in range(NCORES):
        r, half = c // 2, c % 2
        s_all, d_all, isloop, counts = structs[c]
        srcr = np.zeros((NW, T * P), np.int64)
        dstr = np.zeros((NW, T * P), np.int64)
        dc12 = np.full((NW, T * P), -1.0, np.float32)
        dc34 = np.full((NW, T * P), -1.0, np.float32)
        start = 0
        off = half * TH if pair_split else 0
        for w in range(NW):
            cnt = counts[w]
            sl = slice(start, start + cnt)
            start += cnt
            srcr[w, :cnt] = glob_row(s_all[sl])
            dstr[w, :cnt] = off + d_all[sl]
            dl = (d_all[sl] - w * P).astype(np.float32)
            dc12[w, :cnt] = dl
            dc34[w, :cnt] = np.where(isloop[sl], -1.0, dl)

        def wrap(a):  # [NW, T*P] -> [NW*P, 8T] int16, replicated per Q7 core
            b = a.astype(np.int16).reshape(NW, 8 * T, 16)
            b = np.transpose(b, (0, 2, 1))
            return np.tile(b, (1, 8, 1)).reshape(NW * P, 8 * T).copy()

        def colmajor(a):  # [NW, T*P] -> [NW*P, T]
            return np.ascontiguousarray(
                np.transpose(a.reshape(NW, T, P), (0, 2, 1))).reshape(
                    NW * P, T)

        kchunk = lambda w: np.ascontiguousarray(
            np.asarray(w[r], np.float32)).reshape(2, P, -1)
        att_rep = lambda a: np.broadcast_to(
            np.asarray(a[r], np.float32).astype(ml_dtypes.bfloat16)
            [None, None], (P, 1, H, HID)).copy()
        brep = lambda b, n: np.broadcast_to(
            np.asarray(b[r], np.float32)[None], (P, n)).copy()
        bf = lambda a: np.ascontiguousarray(a).astype(ml_dtypes.bfloat16)

        in_maps.append(dict(
            src_rows=wrap(srcr),
            src_off=colmajor(srcr).astype(np.int32),
            dstc12=colmajor(dc12).astype(ml_dtypes.bfloat16),
            dstc34=colmajor(dc34).astype(ml_dtypes.bfloat16),
            dstr12=dc12.astype(ml_dtypes.bfloat16),
            xT=xT,
            Wl1=bf(kchunk(Wl1)), Wr1=bf(kchunk(Wr1)),
            Wl2=bf(kchunk(Wl2)), Wr2=bf(kchunk(Wr2)),
            att1=att_rep(att1), att2=att_rep(att2),
            b1r=brep(b1, DIN), b2r=brep(b2, DIN),
            Wrel3=bf(kchunk(Wrel3)), Wroot3=bf(kchunk(Wroot3)),
            Wrel4=bf(np.asarray(Wrel4[r], np.float32)),
            Wroot4=bf(np.asarray(Wroot4[r], np.float32)),
            b3r=brep(b3, HID), b4r=brep(b4, OUT),
            iota=iota_c, iotac=iotac_c, ident=ident_c, identb=identb_c,
        ))
    return in_maps, T


def kernel(x, edge_indices, Wl1, Wr1, att1, b1, Wl2, Wr2, att2, b2,
           Wrel3, Wroot3, b3, Wrel4, Wroot4, b4, pair_split=True):
    in_maps, T = _host_prep(x, edge_indices, Wl1, Wr1, att1, b1, Wl2, Wr2,
                            att2, b2, Wrel3, Wroot3, b3, Wrel4, Wroot4, b4,
                            pair_split)
    import os
    key = (T, pair_split, os.environ.get("KPHASES"), os.environ.get("KEDGE"),
           os.environ.get("KNOCC"), os.environ.get("KREP"))
    if key not in _CACHE:
        _CACHE[key] = _build_nc(T, pair_split)
    nc = _CACHE[key]

    res = run_bass_kernel_spmd(nc, in_maps, core_ids=list(range(NCORES)))

    nh, NW, TH, TT = _cfg(pair_split)
    outp = np.zeros((N, R, OUT), np.float32)
    for c in range(NCORES):
        r, half = c // 2, c % 2
        o = res.results[c]["out"]
        if pair_split:
            outp[half * nh:(half + 1) * nh, r] = o[:nh]
        elif half == 0:
            outp[:, r] = o[:N]
    return outp
